# revision 2
# baseline (speedup 1.0000x reference)
"""GCN message-passing kernel for Trainium2 (8 NeuronCores, SPMD).

Math (matches the reference):
    gf   = RF @ W_g                          (2048, 3)   gate features
    H_k  = RF @ W_k                          (2048, 4096) per edge type k in {0,1,2}
    gate(e) = sigmoid(gf[src_e, k_e] + b_glab[p_e])
    upd[t]  = sum_{e->t} gate(e) * (H_{k_e}[src_e] + b_lab[p_e])
    out  = relu(upd)

Because every image's graph is self-contained (32 regions/image) the
edge aggregation is a block-diagonal linear operator: with 4 images per
128-row block,
    upd = sum_k M_k @ H_k + G @ b_lab
where M_k are (128x128)-block-diagonal gate matrices and G is (2048,81).
M_k / G are built ON DEVICE from gf with one-hot constant matrices (host
only prepares 0/1 index matrices), so all data-dependent FLOPs run on
Trainium.

Sharding: the output D dim (4096) is split 8 ways -> each core computes
all 2048 rows x its 512 columns, holding a (4096 x 3*512) slice of
W_conv.  This avoids replicating the 201MB W_conv read (per-core DMA is
~36MB vs ~210MB for image-sharding).  No collectives needed; host
concatenates the column slices.
"""

import numpy as np
import ml_dtypes

# problem constants (hardcoded per contract)
N_IMG = 64
REG = 32
RPI = 32
NUM_REL = 20
D = 4096
NPRED = 81
N = N_IMG * REG          # 2048
NCORES = 8
CW = D // NCORES         # 512 output cols per core
NBLK = N // 128          # 16 row blocks
IPB = 128 // REG         # 4 images per block
EPB = IPB * NUM_REL      # 80 edges per block per edge type

BF = ml_dtypes.bfloat16

_prog_cache = {}


def _build_program():
    import concourse.bass as bass
    import concourse.tile as tile
    from concourse import bacc, mybir

    bf16 = mybir.dt.bfloat16
    f32 = mybir.dt.float32
    AF = mybir.ActivationFunctionType
    ALU = mybir.AluOpType

    nc = bacc.Bacc("TRN2", target_bir_lowering=False, debug=False,
                   num_devices=NCORES)

    rft = nc.dram_tensor("rft", [NBLK, 128, 32 * 128], bf16, kind="ExternalInput").ap()
    w = nc.dram_tensor("w", [128, 3 * 32 * CW], bf16, kind="ExternalInput").ap()
    wg = nc.dram_tensor("wg", [128, 32 * 3], bf16, kind="ExternalInput").ap()
    blab = nc.dram_tensor("blab", [NPRED, CW], bf16, kind="ExternalInput").ap()
    bgb = nc.dram_tensor("bgb", [128, NPRED], bf16, kind="ExternalInput").ap()
    srct = nc.dram_tensor("srct", [128, NBLK * 2 * EPB], bf16, kind="ExternalInput").ap()
    srco = nc.dram_tensor("srco", [EPB, NBLK * 2 * 128], bf16, kind="ExternalInput").ap()
    tgto = nc.dram_tensor("tgto", [EPB, NBLK * 2 * 128], bf16, kind="ExternalInput").ap()
    p1h = nc.dram_tensor("p1h", [EPB, NBLK * NPRED], bf16, kind="ExternalInput").ap()
    p1hs = nc.dram_tensor("p1hs", [128, NPRED], bf16, kind="ExternalInput").ap()
    ident = nc.dram_tensor("ident", [128, 128], bf16, kind="ExternalInput").ap()
    out = nc.dram_tensor("out", [NBLK, 128, CW], f32, kind="ExternalOutput").ap()

    with tile.TileContext(nc) as tc:
        with (
            tc.tile_pool(name="consts", bufs=1) as cpool,
            tc.tile_pool(name="rft", bufs=2) as rpool,
            tc.tile_pool(name="hsb", bufs=2) as hpool,
            tc.tile_pool(name="small", bufs=2) as spool,
            tc.tile_pool(name="osb", bufs=2) as opool,
            tc.tile_pool(name="ph", bufs=2, space="PSUM") as php,
            tc.tile_pool(name="pgf", bufs=1, space="PSUM") as pgfp,
            tc.tile_pool(name="prg", bufs=2, space="PSUM") as prgp,
            tc.tile_pool(name="pgt", bufs=1, space="PSUM") as pgtp,
            tc.tile_pool(name="pmt", bufs=1, space="PSUM") as pmtp,
            tc.tile_pool(name="pout", bufs=1, space="PSUM") as poutp,
        ):
            w_sb = cpool.tile([128, 3 * 32 * CW], bf16, tag="w")
            nc.sync.dma_start(out=w_sb[:], in_=w[:])
            wg_sb = cpool.tile([128, 32 * 3], bf16, tag="wg")
            nc.sync.dma_start(out=wg_sb[:], in_=wg[:])
            blab_sb = cpool.tile([NPRED, CW], bf16, tag="blab")
            nc.sync.dma_start(out=blab_sb[:], in_=blab[:])
            bgb_sb = cpool.tile([128, NPRED], bf16, tag="bgb")
            nc.sync.dma_start(out=bgb_sb[:], in_=bgb[:])
            srct_sb = cpool.tile([128, NBLK * 2 * EPB], bf16, tag="srct")
            nc.sync.dma_start(out=srct_sb[:], in_=srct[:])
            srco_sb = cpool.tile([EPB, NBLK * 2 * 128], bf16, tag="srco")
            nc.sync.dma_start(out=srco_sb[:], in_=srco[:])
            tgto_sb = cpool.tile([EPB, NBLK * 2 * 128], bf16, tag="tgto")
            nc.sync.dma_start(out=tgto_sb[:], in_=tgto[:])
            p1h_sb = cpool.tile([EPB, NBLK * NPRED], bf16, tag="p1h")
            nc.sync.dma_start(out=p1h_sb[:], in_=p1h[:])
            p1hs_sb = cpool.tile([128, NPRED], bf16, tag="p1hs")
            nc.sync.dma_start(out=p1hs_sb[:], in_=p1hs[:])
            ident_sb = cpool.tile([128, 128], bf16, tag="ident")
            nc.sync.dma_start(out=ident_sb[:], in_=ident[:])

            for b in range(NBLK):
                rft_t = rpool.tile([128, 32 * 128], bf16, tag="rft")
                nc.sync.dma_start(out=rft_t[:], in_=rft[b])

                # ---- stage 1: H_k = RF @ W_k  (+ gf on the k=0 pass) ----
                h_sb = []
                pgf_t = pgfp.tile([128, 3], f32, tag="pgf")
                for k in range(3):
                    ph_t = php.tile([128, CW], f32, tag="ph")
                    for d in range(32):
                        lhsT = rft_t[:, d * 128:(d + 1) * 128]
                        nc.tensor.matmul(
                            ph_t[:], lhsT,
                            w_sb[:, (k * 32 + d) * CW:(k * 32 + d + 1) * CW],
                            start=(d == 0), stop=(d == 31),
                        )
                        if k == 0:
                            nc.tensor.matmul(
                                pgf_t[:], lhsT,
                                wg_sb[:, d * 3:(d + 1) * 3],
                                start=(d == 0), stop=(d == 31),
                            )
                    hk = hpool.tile([128, CW], bf16, tag=f"h{k}")
                    nc.vector.tensor_copy(out=hk[:], in_=ph_t[:])
                    h_sb.append(hk)

                gf_sb = spool.tile([128, 3], f32, tag="gf")
                nc.vector.tensor_copy(out=gf_sb[:], in_=pgf_t[:])

                # ---- stage 2: gates -> block-diagonal M_k and G ----
                sig = []
                for k in range(2):
                    sg = spool.tile([128, NPRED], bf16, tag=f"sig{k}")
                    nc.scalar.activation(sg[:], bgb_sb[:], AF.Sigmoid,
                                         bias=gf_sb[:, k:k + 1])
                    sig.append(sg)
                g2 = spool.tile([128, 1], f32, tag="g2")
                nc.scalar.activation(g2[:], bgb_sb[:, 0:1], AF.Sigmoid,
                                     bias=gf_sb[:, 2:3])

                mt_sb = spool.tile([128, 3 * 128], bf16, tag="mt")
                pgt_t = pgtp.tile([NPRED, 128], f32, tag="pgt")
                for k in range(2):
                    prg_t = prgp.tile([EPB, NPRED], f32, tag="prg")
                    nc.tensor.matmul(
                        prg_t[:],
                        srct_sb[:, (b * 2 + k) * EPB:(b * 2 + k + 1) * EPB],
                        sig[k][:], start=True, stop=True)
                    pg = spool.tile([EPB, NPRED], bf16, tag="pg")
                    nc.vector.tensor_mul(
                        pg[:], prg_t[:],
                        p1h_sb[:, b * NPRED:(b + 1) * NPRED])
                    nc.tensor.matmul(
                        pgt_t[:], pg[:],
                        tgto_sb[:, (b * 2 + k) * 128:(b * 2 + k + 1) * 128],
                        start=(k == 0), stop=False)
                    gcol = spool.tile([EPB, 1], f32, tag="gcol")
                    nc.vector.tensor_reduce(gcol[:], pg[:],
                                            axis=mybir.AxisListType.X,
                                            op=ALU.add)
                    srcg = spool.tile([EPB, 128], bf16, tag="srcg")
                    nc.vector.tensor_scalar_mul(
                        srcg[:],
                        srco_sb[:, (b * 2 + k) * 128:(b * 2 + k + 1) * 128],
                        gcol[:])
                    pmt_t = pmtp.tile([128, 128], f32, tag="pmt")
                    nc.tensor.matmul(
                        pmt_t[:], srcg[:],
                        tgto_sb[:, (b * 2 + k) * 128:(b * 2 + k + 1) * 128],
                        start=True, stop=True)
                    nc.vector.tensor_copy(out=mt_sb[:, k * 128:(k + 1) * 128],
                                          in_=pmt_t[:])
                # self-loop: M_2 = diag(g2); G row 0 += g2
                pg2 = spool.tile([128, NPRED], bf16, tag="pg2")
                nc.vector.tensor_scalar_mul(pg2[:], p1hs_sb[:], g2[:])
                nc.tensor.matmul(pgt_t[:], pg2[:], ident_sb[:],
                                 start=False, stop=True)
                gt_sb = spool.tile([NPRED, 128], bf16, tag="gt")
                nc.vector.tensor_copy(out=gt_sb[:], in_=pgt_t[:])
                nc.vector.tensor_scalar_mul(mt_sb[:, 2 * 128:3 * 128],
                                            ident_sb[:], g2[:])

                # ---- stage 3: upd = sum_k M_k @ H_k + G @ b_lab; relu ----
                pout_t = poutp.tile([128, CW], f32, tag="pout")
                for k in range(3):
                    nc.tensor.matmul(pout_t[:],
                                     mt_sb[:, k * 128:(k + 1) * 128],
                                     h_sb[k][:],
                                     start=(k == 0), stop=False)
                nc.tensor.matmul(pout_t[:], gt_sb[:], blab_sb[:],
                                 start=False, stop=True)
                out_sb = opool.tile([128, CW], f32, tag="out")
                nc.scalar.activation(out_sb[:], pout_t[:], AF.Relu)
                nc.sync.dma_start(out=out[b], in_=out_sb[:])

    nc.compile()
    return nc


def _host_prep(inputs):
    rf = np.asarray(inputs["region_feats"], dtype=np.float32)
    W = np.asarray(inputs["W_conv"], dtype=np.float32)
    Wg = np.asarray(inputs["W_g"], dtype=np.float32)
    blab = np.asarray(inputs["b_lab"], dtype=np.float32)
    bglab = np.asarray(inputs["b_glab"], dtype=np.float32)
    rels = np.asarray(inputs["rels"])
    preds = np.asarray(inputs["pred_classes"])

    rels_r = rels.reshape(N_IMG, RPI, 3)[:, :NUM_REL].reshape(-1, 3)
    preds_r = preds.reshape(N_IMG, RPI)[:, :NUM_REL].reshape(-1)

    # RF^T tiles: rft_h[b, p, d*128+j] = RF[b*128+j, d*128+p]
    rft_h = np.ascontiguousarray(
        rf.T.reshape(32, 128, NBLK, 128).transpose(2, 1, 0, 3), dtype=BF
    ).reshape(NBLK, 128, 32 * 128)

    # W slices per core: w_h[p, ((k*32+d)*CW)+j] = W[d*128+p, k*D + c*CW + j]
    Wr = W.reshape(32, 128, 3, NCORES, CW)
    w_cores = [
        np.ascontiguousarray(Wr[:, :, :, c, :].transpose(1, 2, 0, 3),
                             dtype=BF).reshape(128, 3 * 32 * CW)
        for c in range(NCORES)
    ]
    wg_h = np.ascontiguousarray(
        Wg.reshape(32, 128, 3).transpose(1, 0, 2), dtype=BF
    ).reshape(128, 32 * 3)
    blab_cores = [
        np.ascontiguousarray(blab[:, c * CW:(c + 1) * CW], dtype=BF)
        for c in range(NCORES)
    ]
    bgb_h = np.ascontiguousarray(
        np.repeat(bglab.reshape(1, NPRED), 128, axis=0), dtype=BF)

    srct_h = np.zeros((128, NBLK * 2 * EPB), np.float32)
    srco_h = np.zeros((EPB, NBLK * 2 * 128), np.float32)
    tgto_h = np.zeros((EPB, NBLK * 2 * 128), np.float32)
    p1h_h = np.zeros((EPB, NBLK * NPRED), np.float32)
    e = np.arange(EPB)
    for b in range(NBLK):
        eb = rels_r[b * EPB:(b + 1) * EPB]
        pb = preds_r[b * EPB:(b + 1) * EPB]
        s = eb[:, 1] - b * 128
        o = eb[:, 2] - b * 128
        # k=0: obj -> subj (src=o, tgt=s); k=1: subj -> obj (src=s, tgt=o)
        srct_h[o, (b * 2 + 0) * EPB + e] = 1.0
        srct_h[s, (b * 2 + 1) * EPB + e] = 1.0
        srco_h[e, (b * 2 + 0) * 128 + o] = 1.0
        srco_h[e, (b * 2 + 1) * 128 + s] = 1.0
        tgto_h[e, (b * 2 + 0) * 128 + s] = 1.0
        tgto_h[e, (b * 2 + 1) * 128 + o] = 1.0
        p1h_h[e, b * NPRED + pb] = 1.0
    p1hs_h = np.zeros((128, NPRED), np.float32)
    p1hs_h[:, 0] = 1.0

    shared = {
        "rft": rft_h,
        "wg": wg_h,
        "bgb": bgb_h,
        "srct": srct_h.astype(BF),
        "srco": srco_h.astype(BF),
        "tgto": tgto_h.astype(BF),
        "p1h": p1h_h.astype(BF),
        "p1hs": p1hs_h.astype(BF),
        "ident": np.eye(128, dtype=np.float32).astype(BF),
    }
    in_maps = []
    for c in range(NCORES):
        m = dict(shared)
        m["w"] = w_cores[c]
        m["blab"] = blab_cores[c]
        in_maps.append(m)
    return in_maps


def _rels_are_blocked(rels):
    """Check each image's relations reference only that image's regions."""
    rels = np.asarray(rels)
    if rels.shape != (N_IMG * RPI, 3):
        return False
    rels_r = rels.reshape(N_IMG, RPI, 3)[:, :NUM_REL]
    img = np.arange(N_IMG)[:, None]
    lo, hi = img * REG, (img + 1) * REG
    so = rels_r[:, :, 1:3]
    return bool(np.all((so >= lo[:, :, None]) & (so < hi[:, :, None])))


def _numpy_fallback(inputs):
    """Reference-equivalent host computation (only used if the per-image
    relation structure assumption is violated)."""
    rf = np.asarray(inputs["region_feats"], dtype=np.float32)
    W = np.asarray(inputs["W_conv"], dtype=np.float32)
    Wg = np.asarray(inputs["W_g"], dtype=np.float32)
    blab = np.asarray(inputs["b_lab"], dtype=np.float32)
    bglab = np.asarray(inputs["b_glab"], dtype=np.float32)
    rels = np.asarray(inputs["rels"])
    preds = np.asarray(inputs["pred_classes"])
    rels_r = rels.reshape(N_IMG, RPI, 3)[:, :NUM_REL].reshape(-1, 3)
    preds_r = preds.reshape(N_IMG, RPI)[:, :NUM_REL].reshape(-1)
    nf = (rf @ W).reshape(-1, D)
    gfe = (rf @ Wg).reshape(-1)
    s, o = rels_r[:, 1], rels_r[:, 2]
    self_ids = np.arange(N)
    idx = np.concatenate([o * 3 + 0, s * 3 + 1, self_ids * 3 + 2])
    pr = np.concatenate([preds_r, preds_r, np.zeros(N, preds_r.dtype)])
    tgt = np.concatenate([s, o, self_ids])
    gate = 1.0 / (1.0 + np.exp(-(gfe[idx] + bglab[pr, 0])))
    msg = gate[:, None] * (nf[idx] + blab[pr])
    upd = np.zeros((N, D), np.float32)
    np.add.at(upd, tgt, msg)
    return np.maximum(upd, 0.0)


def _run(inputs, trace=False):
    from concourse.bass_utils import run_bass_kernel_spmd

    if "nc" not in _prog_cache:
        _prog_cache["nc"] = _build_program()
    nc = _prog_cache["nc"]
    in_maps = _host_prep(inputs)
    res = run_bass_kernel_spmd(nc, in_maps, core_ids=list(range(NCORES)),
                               trace=trace)
    out = np.empty((N, D), np.float32)
    for c in range(NCORES):
        out[:, c * CW:(c + 1) * CW] = (
            np.asarray(res.results[c]["out"]).reshape(N, CW))
    return out, res


def kernel(**inputs):
    if not _rels_are_blocked(inputs["rels"]):
        return _numpy_fallback(inputs)
    out, _ = _run(inputs, trace=False)
    return out


# revision 7
# speedup vs baseline: 1.0351x; 1.0351x over previous
"""GCN message-passing kernel for Trainium2 (8 NeuronCores, SPMD).

Math (matches the reference):
    gf   = RF @ W_g                          (2048, 3)   gate features
    H_k  = RF @ W_k                          (2048, 4096) per edge type k in {0,1,2}
    gate(e) = sigmoid(gf[src_e, k_e] + b_glab[p_e])
    upd[t]  = sum_{e->t} gate(e) * (H_{k_e}[src_e] + b_lab[p_e])
    out  = relu(upd)

Because every image's graph is self-contained (32 regions/image) the
edge aggregation is a block-diagonal linear operator: with 4 images per
128-row block,
    upd = sum_k M_k @ H_k + G @ b_lab
where M_k are (128x128)-block-diagonal gate matrices and G is (2048,81).
M_k / G are built ON DEVICE from gf with one-hot constant matrices (host
only prepares 0/1 index matrices), so all data-dependent FLOPs run on
Trainium.

Sharding: the output D dim (4096) is split 8 ways -> each core computes
all 2048 rows x its 512 columns, holding a (4096 x 3*512) slice of
W_conv.  This avoids replicating the 201MB W_conv read (per-core DMA is
~36MB vs ~210MB for image-sharding).  No collectives needed; host
concatenates the column slices.
"""

import numpy as np
import ml_dtypes

# problem constants (hardcoded per contract)
N_IMG = 64
REG = 32
RPI = 32
NUM_REL = 20
D = 4096
NPRED = 81
N = N_IMG * REG          # 2048
NCORES = 8
CW = D // NCORES         # 512 output cols per core
NBLK = N // 128          # 16 row blocks
IPB = 128 // REG         # 4 images per block
EPB = IPB * NUM_REL      # 80 edges per block per edge type

BF = ml_dtypes.bfloat16

_prog_cache = {}


def _build_program():
    import concourse.bass as bass
    import concourse.tile as tile
    from concourse import bacc, mybir

    bf16 = mybir.dt.bfloat16
    f32 = mybir.dt.float32
    AF = mybir.ActivationFunctionType
    ALU = mybir.AluOpType

    nc = bacc.Bacc("TRN2", target_bir_lowering=False, debug=False,
                   num_devices=NCORES)

    rft = nc.dram_tensor("rft", [NBLK, 128, 32 * 128], bf16, kind="ExternalInput").ap()
    w = nc.dram_tensor("w", [128, 3 * 32 * CW], bf16, kind="ExternalInput").ap()
    wg = nc.dram_tensor("wg", [128, 32 * 3], bf16, kind="ExternalInput").ap()
    blab = nc.dram_tensor("blab", [NPRED, CW], bf16, kind="ExternalInput").ap()
    bgb = nc.dram_tensor("bgb", [128, NPRED], bf16, kind="ExternalInput").ap()
    srct = nc.dram_tensor("srct", [128, NBLK * 2 * EPB], bf16, kind="ExternalInput").ap()
    srco = nc.dram_tensor("srco", [EPB, NBLK * 2 * 128], bf16, kind="ExternalInput").ap()
    tgto = nc.dram_tensor("tgto", [EPB, NBLK * 2 * 128], bf16, kind="ExternalInput").ap()
    p1h = nc.dram_tensor("p1h", [EPB, NBLK * NPRED], bf16, kind="ExternalInput").ap()
    p1hs = nc.dram_tensor("p1hs", [128, NPRED], bf16, kind="ExternalInput").ap()
    ident = nc.dram_tensor("ident", [128, 128], bf16, kind="ExternalInput").ap()
    out = nc.dram_tensor("out", [NBLK, 128, CW], f32, kind="ExternalOutput").ap()

    with tile.TileContext(nc) as tc:
        with (
            tc.tile_pool(name="consts", bufs=1) as cpool,
            tc.tile_pool(name="rft", bufs=2) as rpool,
            tc.tile_pool(name="hsb", bufs=2) as hpool,
            tc.tile_pool(name="small", bufs=2) as spool,
            tc.tile_pool(name="osb", bufs=2) as opool,
            tc.tile_pool(name="ph", bufs=2, space="PSUM") as php,
            tc.tile_pool(name="pgf", bufs=1, space="PSUM") as pgfp,
            tc.tile_pool(name="prg", bufs=2, space="PSUM") as prgp,
            tc.tile_pool(name="pgt", bufs=1, space="PSUM") as pgtp,
            tc.tile_pool(name="pmt", bufs=1, space="PSUM") as pmtp,
            tc.tile_pool(name="pout", bufs=1, space="PSUM") as poutp,
        ):
            # DMA order matters for startup latency: first the tiles block 0
            # needs (rft[0] is issued inside the b-loop; wg + W[k=0] here),
            # then the small gate constants, then the rest of W.
            wg_sb = cpool.tile([128, 32 * 3], bf16, tag="wg")
            nc.sync.dma_start(out=wg_sb[:], in_=wg[:])
            w_sb_k = []
            for k in range(3):
                wk = cpool.tile([128, 32 * CW], bf16, tag=f"w{k}")
                w_sb_k.append(wk)
            nc.sync.dma_start(out=w_sb_k[0][:],
                              in_=w[:, 0 * 32 * CW:1 * 32 * CW])
            rft_cur = rpool.tile([128, 32 * 128], bf16, tag="rft")
            nc.sync.dma_start(out=rft_cur[:], in_=rft[0])
            blab_sb = cpool.tile([NPRED, CW], bf16, tag="blab")
            nc.sync.dma_start(out=blab_sb[:], in_=blab[:])
            bgb_sb = cpool.tile([128, NPRED], bf16, tag="bgb")
            nc.sync.dma_start(out=bgb_sb[:], in_=bgb[:])
            srct_sb = cpool.tile([128, NBLK * 2 * EPB], bf16, tag="srct")
            nc.sync.dma_start(out=srct_sb[:], in_=srct[:])
            srco_sb = cpool.tile([EPB, NBLK * 2 * 128], bf16, tag="srco")
            nc.sync.dma_start(out=srco_sb[:], in_=srco[:])
            tgto_sb = cpool.tile([EPB, NBLK * 2 * 128], bf16, tag="tgto")
            nc.sync.dma_start(out=tgto_sb[:], in_=tgto[:])
            p1h_sb = cpool.tile([EPB, NBLK * NPRED], bf16, tag="p1h")
            nc.sync.dma_start(out=p1h_sb[:], in_=p1h[:])
            p1hs_sb = cpool.tile([128, NPRED], bf16, tag="p1hs")
            nc.sync.dma_start(out=p1hs_sb[:], in_=p1hs[:])
            ident_sb = cpool.tile([128, 128], bf16, tag="ident")
            nc.sync.dma_start(out=ident_sb[:], in_=ident[:])
            for k in (1, 2):
                nc.sync.dma_start(out=w_sb_k[k][:],
                                  in_=w[:, k * 32 * CW:(k + 1) * 32 * CW])

            for b in range(NBLK):
                rft_t = rft_cur
                if b + 1 < NBLK:
                    rft_cur = rpool.tile([128, 32 * 128], bf16, tag="rft")
                    nc.sync.dma_start(out=rft_cur[:], in_=rft[b + 1])

                # ---- stage 1: H_k = RF @ W_k  (+ gf on the k=0 pass) ----
                h_sb = []
                pgf_t = pgfp.tile([128, 3], f32, tag="pgf")
                for k in range(3):
                    ph_t = php.tile([128, CW], f32, tag="ph")
                    for d in range(32):
                        lhsT = rft_t[:, d * 128:(d + 1) * 128]
                        nc.tensor.matmul(
                            ph_t[:], lhsT,
                            w_sb_k[k][:, d * CW:(d + 1) * CW],
                            start=(d == 0), stop=(d == 31),
                        )
                        if k == 0:
                            nc.tensor.matmul(
                                pgf_t[:], lhsT,
                                wg_sb[:, d * 3:(d + 1) * 3],
                                start=(d == 0), stop=(d == 31),
                            )
                    hk = hpool.tile([128, CW], bf16, tag=f"h{k}")
                    nc.vector.tensor_copy(out=hk[:], in_=ph_t[:])
                    h_sb.append(hk)

                gf_sb = spool.tile([128, 3], f32, tag="gf")
                nc.vector.tensor_copy(out=gf_sb[:], in_=pgf_t[:])

                # ---- stage 2: gates -> block-diagonal M_k and G ----
                sig = []
                for k in range(2):
                    sg = spool.tile([128, NPRED], bf16, tag=f"sig{k}")
                    nc.scalar.activation(sg[:], bgb_sb[:], AF.Sigmoid,
                                         bias=gf_sb[:, k:k + 1])
                    sig.append(sg)
                g2 = spool.tile([128, 1], f32, tag="g2")
                nc.scalar.activation(g2[:], bgb_sb[:, 0:1], AF.Sigmoid,
                                     bias=gf_sb[:, 2:3])

                mt_sb = spool.tile([128, 3 * 128], bf16, tag="mt")
                pgt_t = pgtp.tile([NPRED, 128], f32, tag="pgt")
                for k in range(2):
                    prg_t = prgp.tile([EPB, NPRED], f32, tag="prg")
                    nc.tensor.matmul(
                        prg_t[:],
                        srct_sb[:, (b * 2 + k) * EPB:(b * 2 + k + 1) * EPB],
                        sig[k][:], start=True, stop=True)
                    pg = spool.tile([EPB, NPRED], bf16, tag="pg")
                    nc.vector.tensor_mul(
                        pg[:], prg_t[:],
                        p1h_sb[:, b * NPRED:(b + 1) * NPRED])
                    nc.tensor.matmul(
                        pgt_t[:], pg[:],
                        tgto_sb[:, (b * 2 + k) * 128:(b * 2 + k + 1) * 128],
                        start=(k == 0), stop=False)
                    gcol = spool.tile([EPB, 1], f32, tag="gcol")
                    nc.vector.tensor_reduce(gcol[:], pg[:],
                                            axis=mybir.AxisListType.X,
                                            op=ALU.add)
                    srcg = spool.tile([EPB, 128], bf16, tag="srcg")
                    nc.vector.tensor_scalar_mul(
                        srcg[:],
                        srco_sb[:, (b * 2 + k) * 128:(b * 2 + k + 1) * 128],
                        gcol[:])
                    pmt_t = pmtp.tile([128, 128], f32, tag="pmt")
                    nc.tensor.matmul(
                        pmt_t[:], srcg[:],
                        tgto_sb[:, (b * 2 + k) * 128:(b * 2 + k + 1) * 128],
                        start=True, stop=True)
                    nc.vector.tensor_copy(out=mt_sb[:, k * 128:(k + 1) * 128],
                                          in_=pmt_t[:])
                # self-loop: M_2 = diag(g2); G row 0 += g2
                pg2 = spool.tile([128, NPRED], bf16, tag="pg2")
                nc.vector.tensor_scalar_mul(pg2[:], p1hs_sb[:], g2[:])
                nc.tensor.matmul(pgt_t[:], pg2[:], ident_sb[:],
                                 start=False, stop=True)
                gt_sb = spool.tile([NPRED, 128], bf16, tag="gt")
                nc.vector.tensor_copy(out=gt_sb[:], in_=pgt_t[:])
                nc.vector.tensor_scalar_mul(mt_sb[:, 2 * 128:3 * 128],
                                            ident_sb[:], g2[:])

                # ---- stage 3: upd = sum_k M_k @ H_k + G @ b_lab; relu ----
                pout_t = poutp.tile([128, CW], f32, tag="pout")
                for k in range(3):
                    nc.tensor.matmul(pout_t[:],
                                     mt_sb[:, k * 128:(k + 1) * 128],
                                     h_sb[k][:],
                                     start=(k == 0), stop=False)
                nc.tensor.matmul(pout_t[:], gt_sb[:], blab_sb[:],
                                 start=False, stop=True)
                out_sb = opool.tile([128, CW], f32, tag="out")
                nc.scalar.activation(out_sb[:], pout_t[:], AF.Relu)
                nc.sync.dma_start(out=out[b], in_=out_sb[:])

    nc.compile()
    return nc


def _host_prep(inputs):
    rf = np.asarray(inputs["region_feats"], dtype=np.float32)
    W = np.asarray(inputs["W_conv"], dtype=np.float32)
    Wg = np.asarray(inputs["W_g"], dtype=np.float32)
    blab = np.asarray(inputs["b_lab"], dtype=np.float32)
    bglab = np.asarray(inputs["b_glab"], dtype=np.float32)
    rels = np.asarray(inputs["rels"])
    preds = np.asarray(inputs["pred_classes"])

    rels_r = rels.reshape(N_IMG, RPI, 3)[:, :NUM_REL].reshape(-1, 3)
    preds_r = preds.reshape(N_IMG, RPI)[:, :NUM_REL].reshape(-1)

    # RF^T tiles: rft_h[b, p, d*128+j] = RF[b*128+j, d*128+p]
    rft_h = np.ascontiguousarray(
        rf.T.reshape(32, 128, NBLK, 128).transpose(2, 1, 0, 3), dtype=BF
    ).reshape(NBLK, 128, 32 * 128)

    # W slices per core: w_h[p, ((k*32+d)*CW)+j] = W[d*128+p, k*D + c*CW + j]
    Wr = W.reshape(32, 128, 3, NCORES, CW)
    w_cores = [
        np.ascontiguousarray(Wr[:, :, :, c, :].transpose(1, 2, 0, 3),
                             dtype=BF).reshape(128, 3 * 32 * CW)
        for c in range(NCORES)
    ]
    wg_h = np.ascontiguousarray(
        Wg.reshape(32, 128, 3).transpose(1, 0, 2), dtype=BF
    ).reshape(128, 32 * 3)
    blab_cores = [
        np.ascontiguousarray(blab[:, c * CW:(c + 1) * CW], dtype=BF)
        for c in range(NCORES)
    ]
    bgb_h = np.ascontiguousarray(
        np.repeat(bglab.reshape(1, NPRED), 128, axis=0), dtype=BF)

    srct_h = np.zeros((128, NBLK * 2 * EPB), np.float32)
    srco_h = np.zeros((EPB, NBLK * 2 * 128), np.float32)
    tgto_h = np.zeros((EPB, NBLK * 2 * 128), np.float32)
    p1h_h = np.zeros((EPB, NBLK * NPRED), np.float32)
    e = np.arange(EPB)
    for b in range(NBLK):
        eb = rels_r[b * EPB:(b + 1) * EPB]
        pb = preds_r[b * EPB:(b + 1) * EPB]
        s = eb[:, 1] - b * 128
        o = eb[:, 2] - b * 128
        # k=0: obj -> subj (src=o, tgt=s); k=1: subj -> obj (src=s, tgt=o)
        srct_h[o, (b * 2 + 0) * EPB + e] = 1.0
        srct_h[s, (b * 2 + 1) * EPB + e] = 1.0
        srco_h[e, (b * 2 + 0) * 128 + o] = 1.0
        srco_h[e, (b * 2 + 1) * 128 + s] = 1.0
        tgto_h[e, (b * 2 + 0) * 128 + s] = 1.0
        tgto_h[e, (b * 2 + 1) * 128 + o] = 1.0
        p1h_h[e, b * NPRED + pb] = 1.0
    p1hs_h = np.zeros((128, NPRED), np.float32)
    p1hs_h[:, 0] = 1.0

    shared = {
        "rft": rft_h,
        "wg": wg_h,
        "bgb": bgb_h,
        "srct": srct_h.astype(BF),
        "srco": srco_h.astype(BF),
        "tgto": tgto_h.astype(BF),
        "p1h": p1h_h.astype(BF),
        "p1hs": p1hs_h.astype(BF),
        "ident": np.eye(128, dtype=np.float32).astype(BF),
    }
    in_maps = []
    for c in range(NCORES):
        m = dict(shared)
        m["w"] = w_cores[c]
        m["blab"] = blab_cores[c]
        in_maps.append(m)
    return in_maps


def _rels_are_blocked(rels):
    """Check each image's relations reference only that image's regions."""
    rels = np.asarray(rels)
    if rels.shape != (N_IMG * RPI, 3):
        return False
    rels_r = rels.reshape(N_IMG, RPI, 3)[:, :NUM_REL]
    img = np.arange(N_IMG)[:, None]
    lo, hi = img * REG, (img + 1) * REG
    so = rels_r[:, :, 1:3]
    return bool(np.all((so >= lo[:, :, None]) & (so < hi[:, :, None])))


def _numpy_fallback(inputs):
    """Reference-equivalent host computation (only used if the per-image
    relation structure assumption is violated)."""
    rf = np.asarray(inputs["region_feats"], dtype=np.float32)
    W = np.asarray(inputs["W_conv"], dtype=np.float32)
    Wg = np.asarray(inputs["W_g"], dtype=np.float32)
    blab = np.asarray(inputs["b_lab"], dtype=np.float32)
    bglab = np.asarray(inputs["b_glab"], dtype=np.float32)
    rels = np.asarray(inputs["rels"])
    preds = np.asarray(inputs["pred_classes"])
    rels_r = rels.reshape(N_IMG, RPI, 3)[:, :NUM_REL].reshape(-1, 3)
    preds_r = preds.reshape(N_IMG, RPI)[:, :NUM_REL].reshape(-1)
    nf = (rf @ W).reshape(-1, D)
    gfe = (rf @ Wg).reshape(-1)
    s, o = rels_r[:, 1], rels_r[:, 2]
    self_ids = np.arange(N)
    idx = np.concatenate([o * 3 + 0, s * 3 + 1, self_ids * 3 + 2])
    pr = np.concatenate([preds_r, preds_r, np.zeros(N, preds_r.dtype)])
    tgt = np.concatenate([s, o, self_ids])
    gate = 1.0 / (1.0 + np.exp(-(gfe[idx] + bglab[pr, 0])))
    msg = gate[:, None] * (nf[idx] + blab[pr])
    upd = np.zeros((N, D), np.float32)
    np.add.at(upd, tgt, msg)
    return np.maximum(upd, 0.0)


def _run(inputs, trace=False):
    from concourse.bass_utils import run_bass_kernel_spmd

    if "nc" not in _prog_cache:
        _prog_cache["nc"] = _build_program()
    nc = _prog_cache["nc"]
    in_maps = _host_prep(inputs)
    res = run_bass_kernel_spmd(nc, in_maps, core_ids=list(range(NCORES)),
                               trace=trace)
    out = np.empty((N, D), np.float32)
    for c in range(NCORES):
        out[:, c * CW:(c + 1) * CW] = (
            np.asarray(res.results[c]["out"]).reshape(N, CW))
    return out, res


def kernel(**inputs):
    if not _rels_are_blocked(inputs["rels"]):
        return _numpy_fallback(inputs)
    out, _ = _run(inputs, trace=False)
    return out


# revision 12
# speedup vs baseline: 1.0454x; 1.0100x over previous
"""GCN message-passing kernel for Trainium2 (8 NeuronCores, SPMD).

Math (matches the reference):
    gf   = RF @ W_g                          (2048, 3)   gate features
    H_k  = RF @ W_k                          (2048, 4096) per edge type k in {0,1,2}
    gate(e) = sigmoid(gf[src_e, k_e] + b_glab[p_e])
    upd[t]  = sum_{e->t} gate(e) * (H_{k_e}[src_e] + b_lab[p_e])
    out  = relu(upd)

Because every image's graph is self-contained (32 regions/image) the
edge aggregation is a block-diagonal linear operator: with 4 images per
128-row block,
    upd = sum_k M_k @ H_k + G @ b_lab
where M_k are (128x128)-block-diagonal gate matrices and G is (2048,81).
M_k / G are built ON DEVICE from gf with one-hot constant matrices (host
only prepares 0/1 index matrices), so all data-dependent FLOPs run on
Trainium.

Sharding: the output D dim (4096) is split 8 ways -> each core computes
all 2048 rows x its 512 columns, holding a (4096 x 3*512) slice of
W_conv.  This avoids replicating the 201MB W_conv read (per-core DMA is
~36MB vs ~210MB for image-sharding).  No collectives needed; host
concatenates the column slices.
"""

import numpy as np
import ml_dtypes

# problem constants (hardcoded per contract)
N_IMG = 64
REG = 32
RPI = 32
NUM_REL = 20
D = 4096
NPRED = 81
N = N_IMG * REG          # 2048
NCORES = 8
CW = D // NCORES         # 512 output cols per core
NBLK = N // 128          # 16 row blocks
IPB = 128 // REG         # 4 images per block
EPB = IPB * NUM_REL      # 80 edges per block per edge type

BF = ml_dtypes.bfloat16

_prog_cache = {}


def _build_program():
    import concourse.bass as bass
    import concourse.tile as tile
    from concourse import bacc, mybir

    bf16 = mybir.dt.bfloat16
    f32 = mybir.dt.float32
    AF = mybir.ActivationFunctionType
    ALU = mybir.AluOpType

    nc = bacc.Bacc("TRN2", target_bir_lowering=False, debug=False,
                   num_devices=NCORES)

    rft = nc.dram_tensor("rft", [NBLK, 128, 32 * 128], bf16, kind="ExternalInput").ap()
    w = nc.dram_tensor("w", [128, 3 * 32 * CW], bf16, kind="ExternalInput").ap()
    wg = nc.dram_tensor("wg", [128, 32 * 3], bf16, kind="ExternalInput").ap()
    blab = nc.dram_tensor("blab", [NPRED, CW], bf16, kind="ExternalInput").ap()
    bgb = nc.dram_tensor("bgb", [128, NPRED], bf16, kind="ExternalInput").ap()
    srct = nc.dram_tensor("srct", [128, NBLK * 2 * EPB], bf16, kind="ExternalInput").ap()
    srco = nc.dram_tensor("srco", [EPB, NBLK * 2 * 128], bf16, kind="ExternalInput").ap()
    tgto = nc.dram_tensor("tgto", [EPB, NBLK * 2 * 128], bf16, kind="ExternalInput").ap()
    p1h = nc.dram_tensor("p1h", [EPB, NBLK * NPRED], bf16, kind="ExternalInput").ap()
    p1hs = nc.dram_tensor("p1hs", [128, NPRED], bf16, kind="ExternalInput").ap()
    ident = nc.dram_tensor("ident", [128, 128], bf16, kind="ExternalInput").ap()
    out = nc.dram_tensor("out", [NBLK, 128, CW], f32, kind="ExternalOutput").ap()

    with tile.TileContext(nc) as tc:
        with (
            tc.tile_pool(name="consts", bufs=1) as cpool,
            tc.tile_pool(name="rft", bufs=2) as rpool,
            tc.tile_pool(name="hsb", bufs=2) as hpool,
            tc.tile_pool(name="small", bufs=2) as spool,
            tc.tile_pool(name="osb", bufs=2) as opool,
            tc.tile_pool(name="ph", bufs=2, space="PSUM") as php,
            tc.tile_pool(name="pgf", bufs=1, space="PSUM") as pgfp,
            tc.tile_pool(name="prg", bufs=2, space="PSUM") as prgp,
            tc.tile_pool(name="pgt", bufs=1, space="PSUM") as pgtp,
            tc.tile_pool(name="pmt", bufs=1, space="PSUM") as pmtp,
            tc.tile_pool(name="pout", bufs=1, space="PSUM") as poutp,
        ):
            # DMA order matters for startup latency: block 0's first matmuls
            # need wg + the first chunk of W[k=0] + rft[0]'s first half, so
            # those are issued first (W[k=0] split into 4 chunks, rft into
            # halves); the gate constants and W[k=1,2] follow.
            wg_sb = cpool.tile([128, 32 * 3], bf16, tag="wg")
            nc.sync.dma_start(out=wg_sb[:], in_=wg[:])
            w_sb_k = [None]
            for k in (1, 2):
                wk = cpool.tile([128, 32 * CW], bf16, tag=f"w{k}")
                w_sb_k.append(wk)
            WCH = 8 * CW  # w chunk: 8 d-tiles
            w0_ch = [cpool.tile([128, WCH], bf16, tag=f"w0c{i}", name=f"w0c{i}")
                     for i in range(4)]
            nc.sync.dma_start(out=w0_ch[0][:], in_=w[:, 0:WCH])

            def _load_rft(b):
                ra = rpool.tile([128, 16 * 128], bf16, tag="rfta")
                rb = rpool.tile([128, 16 * 128], bf16, tag="rftb")
                nc.sync.dma_start(out=ra[:], in_=rft[b, :, 0:16 * 128])
                nc.sync.dma_start(out=rb[:], in_=rft[b, :, 16 * 128:32 * 128])
                return ra, rb

            rft_cur = _load_rft(0)
            for i in range(1, 4):
                nc.sync.dma_start(out=w0_ch[i][:],
                                  in_=w[:, i * WCH:(i + 1) * WCH])
            blab_sb = cpool.tile([NPRED, CW], bf16, tag="blab")
            nc.sync.dma_start(out=blab_sb[:], in_=blab[:])
            bgb_sb = cpool.tile([128, NPRED], bf16, tag="bgb")
            nc.sync.dma_start(out=bgb_sb[:], in_=bgb[:])
            srct_sb = cpool.tile([128, NBLK * 2 * EPB], bf16, tag="srct")
            nc.sync.dma_start(out=srct_sb[:], in_=srct[:])
            srco_sb = cpool.tile([EPB, NBLK * 2 * 128], bf16, tag="srco")
            nc.sync.dma_start(out=srco_sb[:], in_=srco[:])
            tgto_sb = cpool.tile([EPB, NBLK * 2 * 128], bf16, tag="tgto")
            nc.sync.dma_start(out=tgto_sb[:], in_=tgto[:])
            p1h_sb = cpool.tile([EPB, NBLK * NPRED], bf16, tag="p1h")
            nc.sync.dma_start(out=p1h_sb[:], in_=p1h[:])
            p1hs_sb = cpool.tile([128, NPRED], bf16, tag="p1hs")
            nc.sync.dma_start(out=p1hs_sb[:], in_=p1hs[:])
            ident_sb = cpool.tile([128, 128], bf16, tag="ident")
            nc.sync.dma_start(out=ident_sb[:], in_=ident[:])
            for k in (1, 2):
                nc.sync.dma_start(out=w_sb_k[k][:],
                                  in_=w[:, k * 32 * CW:(k + 1) * 32 * CW])

            for b in range(NBLK):
                rft_ab = rft_cur
                if b + 1 < NBLK:
                    rft_cur = _load_rft(b + 1)

                def rft_lhsT(d):
                    return rft_ab[d // 16][:, (d % 16) * 128:(d % 16 + 1) * 128]

                # ---- stage 1: H_k = RF @ W_k  (+ gf on the k=0 pass) ----
                h_sb = []
                pgf_t = pgfp.tile([128, 3], f32, tag="pgf")
                for k in range(3):
                    ph_t = php.tile([128, CW], f32, tag="ph")
                    for d in range(32):
                        lhsT = rft_lhsT(d)
                        if k == 0:
                            w_rhs = w0_ch[d // 8][:, (d % 8) * CW:(d % 8 + 1) * CW]
                        else:
                            w_rhs = w_sb_k[k][:, d * CW:(d + 1) * CW]
                        nc.tensor.matmul(
                            ph_t[:], lhsT, w_rhs,
                            start=(d == 0), stop=(d == 31),
                        )
                        if k == 0:
                            nc.tensor.matmul(
                                pgf_t[:], lhsT,
                                wg_sb[:, d * 3:(d + 1) * 3],
                                start=(d == 0), stop=(d == 31),
                            )
                    hk = hpool.tile([128, CW], bf16, tag=f"h{k}")
                    nc.vector.tensor_copy(out=hk[:], in_=ph_t[:])
                    h_sb.append(hk)

                gf_sb = spool.tile([128, 3], f32, tag="gf")
                nc.vector.tensor_copy(out=gf_sb[:], in_=pgf_t[:])

                # ---- stage 2: gates -> block-diagonal M_k and G ----
                sig = []
                for k in range(2):
                    sg = spool.tile([128, NPRED], bf16, tag=f"sig{k}")
                    nc.scalar.activation(sg[:], bgb_sb[:], AF.Sigmoid,
                                         bias=gf_sb[:, k:k + 1])
                    sig.append(sg)
                g2 = spool.tile([128, 1], f32, tag="g2")
                nc.scalar.activation(g2[:], bgb_sb[:, 0:1], AF.Sigmoid,
                                     bias=gf_sb[:, 2:3])

                mt_sb = spool.tile([128, 3 * 128], bf16, tag="mt")
                pgt_t = pgtp.tile([NPRED, 128], f32, tag="pgt")
                for k in range(2):
                    prg_t = prgp.tile([EPB, NPRED], f32, tag="prg")
                    nc.tensor.matmul(
                        prg_t[:],
                        srct_sb[:, (b * 2 + k) * EPB:(b * 2 + k + 1) * EPB],
                        sig[k][:], start=True, stop=True)
                    pg = spool.tile([EPB, NPRED], bf16, tag="pg")
                    nc.vector.tensor_mul(
                        pg[:], prg_t[:],
                        p1h_sb[:, b * NPRED:(b + 1) * NPRED])
                    nc.tensor.matmul(
                        pgt_t[:], pg[:],
                        tgto_sb[:, (b * 2 + k) * 128:(b * 2 + k + 1) * 128],
                        start=(k == 0), stop=False)
                    gcol = spool.tile([EPB, 1], f32, tag="gcol")
                    nc.vector.tensor_reduce(gcol[:], pg[:],
                                            axis=mybir.AxisListType.X,
                                            op=ALU.add)
                    srcg = spool.tile([EPB, 128], bf16, tag="srcg")
                    nc.vector.tensor_scalar_mul(
                        srcg[:],
                        srco_sb[:, (b * 2 + k) * 128:(b * 2 + k + 1) * 128],
                        gcol[:])
                    pmt_t = pmtp.tile([128, 128], f32, tag="pmt")
                    nc.tensor.matmul(
                        pmt_t[:], srcg[:],
                        tgto_sb[:, (b * 2 + k) * 128:(b * 2 + k + 1) * 128],
                        start=True, stop=True)
                    nc.vector.tensor_copy(out=mt_sb[:, k * 128:(k + 1) * 128],
                                          in_=pmt_t[:])
                # self-loop: M_2 = diag(g2); G row 0 += g2
                pg2 = spool.tile([128, NPRED], bf16, tag="pg2")
                nc.vector.tensor_scalar_mul(pg2[:], p1hs_sb[:], g2[:])
                nc.tensor.matmul(pgt_t[:], pg2[:], ident_sb[:],
                                 start=False, stop=True)
                gt_sb = spool.tile([NPRED, 128], bf16, tag="gt")
                nc.vector.tensor_copy(out=gt_sb[:], in_=pgt_t[:])
                nc.vector.tensor_scalar_mul(mt_sb[:, 2 * 128:3 * 128],
                                            ident_sb[:], g2[:])

                # ---- stage 3: upd = sum_k M_k @ H_k + G @ b_lab; relu ----
                pout_t = poutp.tile([128, CW], f32, tag="pout")
                for k in range(3):
                    nc.tensor.matmul(pout_t[:],
                                     mt_sb[:, k * 128:(k + 1) * 128],
                                     h_sb[k][:],
                                     start=(k == 0), stop=False)
                nc.tensor.matmul(pout_t[:], gt_sb[:], blab_sb[:],
                                 start=False, stop=True)
                out_sb = opool.tile([128, CW], f32, tag="out")
                nc.scalar.activation(out_sb[:], pout_t[:], AF.Relu)
                nc.sync.dma_start(out=out[b], in_=out_sb[:])

    nc.compile()
    return nc


def _host_prep(inputs):
    rf = np.asarray(inputs["region_feats"], dtype=np.float32)
    W = np.asarray(inputs["W_conv"], dtype=np.float32)
    Wg = np.asarray(inputs["W_g"], dtype=np.float32)
    blab = np.asarray(inputs["b_lab"], dtype=np.float32)
    bglab = np.asarray(inputs["b_glab"], dtype=np.float32)
    rels = np.asarray(inputs["rels"])
    preds = np.asarray(inputs["pred_classes"])

    rels_r = rels.reshape(N_IMG, RPI, 3)[:, :NUM_REL].reshape(-1, 3)
    preds_r = preds.reshape(N_IMG, RPI)[:, :NUM_REL].reshape(-1)

    # RF^T tiles: rft_h[b, p, d*128+j] = RF[b*128+j, d*128+p]
    rft_h = np.ascontiguousarray(
        rf.T.reshape(32, 128, NBLK, 128).transpose(2, 1, 0, 3), dtype=BF
    ).reshape(NBLK, 128, 32 * 128)

    # W slices per core: w_h[p, ((k*32+d)*CW)+j] = W[d*128+p, k*D + c*CW + j]
    Wr = W.reshape(32, 128, 3, NCORES, CW)
    w_cores = [
        np.ascontiguousarray(Wr[:, :, :, c, :].transpose(1, 2, 0, 3),
                             dtype=BF).reshape(128, 3 * 32 * CW)
        for c in range(NCORES)
    ]
    wg_h = np.ascontiguousarray(
        Wg.reshape(32, 128, 3).transpose(1, 0, 2), dtype=BF
    ).reshape(128, 32 * 3)
    blab_cores = [
        np.ascontiguousarray(blab[:, c * CW:(c + 1) * CW], dtype=BF)
        for c in range(NCORES)
    ]
    bgb_h = np.ascontiguousarray(
        np.repeat(bglab.reshape(1, NPRED), 128, axis=0), dtype=BF)

    srct_h = np.zeros((128, NBLK * 2 * EPB), np.float32)
    srco_h = np.zeros((EPB, NBLK * 2 * 128), np.float32)
    tgto_h = np.zeros((EPB, NBLK * 2 * 128), np.float32)
    p1h_h = np.zeros((EPB, NBLK * NPRED), np.float32)
    e = np.arange(EPB)
    for b in range(NBLK):
        eb = rels_r[b * EPB:(b + 1) * EPB]
        pb = preds_r[b * EPB:(b + 1) * EPB]
        s = eb[:, 1] - b * 128
        o = eb[:, 2] - b * 128
        # k=0: obj -> subj (src=o, tgt=s); k=1: subj -> obj (src=s, tgt=o)
        srct_h[o, (b * 2 + 0) * EPB + e] = 1.0
        srct_h[s, (b * 2 + 1) * EPB + e] = 1.0
        srco_h[e, (b * 2 + 0) * 128 + o] = 1.0
        srco_h[e, (b * 2 + 1) * 128 + s] = 1.0
        tgto_h[e, (b * 2 + 0) * 128 + s] = 1.0
        tgto_h[e, (b * 2 + 1) * 128 + o] = 1.0
        p1h_h[e, b * NPRED + pb] = 1.0
    p1hs_h = np.zeros((128, NPRED), np.float32)
    p1hs_h[:, 0] = 1.0

    shared = {
        "rft": rft_h,
        "wg": wg_h,
        "bgb": bgb_h,
        "srct": srct_h.astype(BF),
        "srco": srco_h.astype(BF),
        "tgto": tgto_h.astype(BF),
        "p1h": p1h_h.astype(BF),
        "p1hs": p1hs_h.astype(BF),
        "ident": np.eye(128, dtype=np.float32).astype(BF),
    }
    in_maps = []
    for c in range(NCORES):
        m = dict(shared)
        m["w"] = w_cores[c]
        m["blab"] = blab_cores[c]
        in_maps.append(m)
    return in_maps


def _rels_are_blocked(rels):
    """Check each image's relations reference only that image's regions."""
    rels = np.asarray(rels)
    if rels.shape != (N_IMG * RPI, 3):
        return False
    rels_r = rels.reshape(N_IMG, RPI, 3)[:, :NUM_REL]
    img = np.arange(N_IMG)[:, None]
    lo, hi = img * REG, (img + 1) * REG
    so = rels_r[:, :, 1:3]
    return bool(np.all((so >= lo[:, :, None]) & (so < hi[:, :, None])))


def _numpy_fallback(inputs):
    """Reference-equivalent host computation (only used if the per-image
    relation structure assumption is violated)."""
    rf = np.asarray(inputs["region_feats"], dtype=np.float32)
    W = np.asarray(inputs["W_conv"], dtype=np.float32)
    Wg = np.asarray(inputs["W_g"], dtype=np.float32)
    blab = np.asarray(inputs["b_lab"], dtype=np.float32)
    bglab = np.asarray(inputs["b_glab"], dtype=np.float32)
    rels = np.asarray(inputs["rels"])
    preds = np.asarray(inputs["pred_classes"])
    rels_r = rels.reshape(N_IMG, RPI, 3)[:, :NUM_REL].reshape(-1, 3)
    preds_r = preds.reshape(N_IMG, RPI)[:, :NUM_REL].reshape(-1)
    nf = (rf @ W).reshape(-1, D)
    gfe = (rf @ Wg).reshape(-1)
    s, o = rels_r[:, 1], rels_r[:, 2]
    self_ids = np.arange(N)
    idx = np.concatenate([o * 3 + 0, s * 3 + 1, self_ids * 3 + 2])
    pr = np.concatenate([preds_r, preds_r, np.zeros(N, preds_r.dtype)])
    tgt = np.concatenate([s, o, self_ids])
    gate = 1.0 / (1.0 + np.exp(-(gfe[idx] + bglab[pr, 0])))
    msg = gate[:, None] * (nf[idx] + blab[pr])
    upd = np.zeros((N, D), np.float32)
    np.add.at(upd, tgt, msg)
    return np.maximum(upd, 0.0)


def _run(inputs, trace=False):
    from concourse.bass_utils import run_bass_kernel_spmd

    if "nc" not in _prog_cache:
        _prog_cache["nc"] = _build_program()
    nc = _prog_cache["nc"]
    in_maps = _host_prep(inputs)
    res = run_bass_kernel_spmd(nc, in_maps, core_ids=list(range(NCORES)),
                               trace=trace)
    out = np.empty((N, D), np.float32)
    for c in range(NCORES):
        out[:, c * CW:(c + 1) * CW] = (
            np.asarray(res.results[c]["out"]).reshape(N, CW))
    return out, res


def kernel(**inputs):
    if not _rels_are_blocked(inputs["rels"]):
        return _numpy_fallback(inputs)
    out, _ = _run(inputs, trace=False)
    return out


# revision 15
# speedup vs baseline: 1.0564x; 1.0105x over previous
"""GCN message-passing kernel for Trainium2 (8 NeuronCores, SPMD).

Math (matches the reference):
    gf   = RF @ W_g                          (2048, 3)   gate features
    H_k  = RF @ W_k                          (2048, 4096) per edge type k in {0,1,2}
    gate(e) = sigmoid(gf[src_e, k_e] + b_glab[p_e])
    upd[t]  = sum_{e->t} gate(e) * (H_{k_e}[src_e] + b_lab[p_e])
    out  = relu(upd)

Because every image's graph is self-contained (32 regions/image) the
edge aggregation is a block-diagonal linear operator: with 4 images per
128-row block,
    upd = sum_k M_k @ H_k + G @ b_lab
where M_k are (128x128)-block-diagonal gate matrices and G is (2048,81).
M_k / G are built ON DEVICE from gf with one-hot constant matrices (host
only prepares 0/1 index matrices), so all data-dependent FLOPs run on
Trainium.

Sharding: the output D dim (4096) is split 8 ways -> each core computes
all 2048 rows x its 512 columns, holding a (4096 x 3*512) slice of
W_conv.  This avoids replicating the 201MB W_conv read (per-core DMA is
~36MB vs ~210MB for image-sharding).  No collectives needed; host
concatenates the column slices.
"""

import numpy as np
import ml_dtypes

# problem constants (hardcoded per contract)
N_IMG = 64
REG = 32
RPI = 32
NUM_REL = 20
D = 4096
NPRED = 81
N = N_IMG * REG          # 2048
NCORES = 8
CW = D // NCORES         # 512 output cols per core
NBLK = N // 128          # 16 row blocks
IPB = 128 // REG         # 4 images per block
EPB = IPB * NUM_REL      # 80 edges per block per edge type

BF = ml_dtypes.bfloat16

_prog_cache = {}


def _build_program():
    import concourse.bass as bass
    import concourse.tile as tile
    from concourse import bacc, mybir

    bf16 = mybir.dt.bfloat16
    f32 = mybir.dt.float32
    AF = mybir.ActivationFunctionType
    ALU = mybir.AluOpType

    nc = bacc.Bacc("TRN2", target_bir_lowering=False, debug=False,
                   num_devices=NCORES)

    rft = nc.dram_tensor("rft", [NBLK, 128, 32 * 128], bf16, kind="ExternalInput").ap()
    w = nc.dram_tensor("w", [128, 3 * 32 * CW], bf16, kind="ExternalInput").ap()
    wg = nc.dram_tensor("wg", [128, 32 * 3], bf16, kind="ExternalInput").ap()
    blab = nc.dram_tensor("blab", [NPRED, CW], bf16, kind="ExternalInput").ap()
    bgb = nc.dram_tensor("bgb", [128, NPRED], bf16, kind="ExternalInput").ap()
    srct = nc.dram_tensor("srct", [128, NBLK * 2 * EPB], bf16, kind="ExternalInput").ap()
    srco = nc.dram_tensor("srco", [EPB, NBLK * 2 * 128], bf16, kind="ExternalInput").ap()
    tgto = nc.dram_tensor("tgto", [EPB, NBLK * 2 * 128], bf16, kind="ExternalInput").ap()
    p1h = nc.dram_tensor("p1h", [EPB, NBLK * NPRED], bf16, kind="ExternalInput").ap()
    p1hs = nc.dram_tensor("p1hs", [128, NPRED], bf16, kind="ExternalInput").ap()
    ident = nc.dram_tensor("ident", [128, 128], bf16, kind="ExternalInput").ap()
    out = nc.dram_tensor("out", [NBLK, 128, CW], f32, kind="ExternalOutput").ap()

    GRP = 2  # blocks per software-pipeline group
    with tile.TileContext(nc) as tc:
        with (
            tc.tile_pool(name="consts", bufs=1) as cpool,
            tc.tile_pool(name="rft", bufs=2 * GRP) as rpool,
            tc.tile_pool(name="hsb", bufs=GRP) as hpool,
            tc.tile_pool(name="small", bufs=2) as spool,
            tc.tile_pool(name="osb", bufs=2) as opool,
            tc.tile_pool(name="ph", bufs=2, space="PSUM") as php,
            tc.tile_pool(name="pgf", bufs=1, space="PSUM") as pgfp,
            tc.tile_pool(name="prg", bufs=2, space="PSUM") as prgp,
            tc.tile_pool(name="pgt", bufs=1, space="PSUM") as pgtp,
            tc.tile_pool(name="pmt", bufs=1, space="PSUM") as pmtp,
            tc.tile_pool(name="pout", bufs=1, space="PSUM") as poutp,
        ):
            # DMA order matters for startup latency: block 0's first matmuls
            # need wg + the first chunks of W[k=0] + rft[0]'s first half, so
            # those are issued first (W[k=0] split into 8 chunks, rft into
            # halves); the gate constants and W[k=1,2] follow.
            wg_sb = cpool.tile([128, 32 * 3], bf16, tag="wg")
            nc.sync.dma_start(out=wg_sb[:], in_=wg[:])
            w_sb_k = [None]
            for k in (1, 2):
                wk = cpool.tile([128, 32 * CW], bf16, tag=f"w{k}")
                w_sb_k.append(wk)
            WCH = 4 * CW  # w0 chunk: 4 d-tiles
            w0_ch = [cpool.tile([128, WCH], bf16, tag=f"w0c{i}", name=f"w0c{i}")
                     for i in range(8)]
            nc.sync.dma_start(out=w0_ch[0][:], in_=w[:, 0:WCH])

            def _load_rft(b):
                ra = rpool.tile([128, 16 * 128], bf16, tag="rfta",
                                name=f"rfta{b}")
                rb = rpool.tile([128, 16 * 128], bf16, tag="rftb",
                                name=f"rftb{b}")
                nc.sync.dma_start(out=ra[:], in_=rft[b, :, 0:16 * 128])
                nc.sync.dma_start(out=rb[:], in_=rft[b, :, 16 * 128:32 * 128])
                return ra, rb

            rft_tiles = {b: _load_rft(b) for b in range(GRP)}
            for i in range(1, 8):
                nc.sync.dma_start(out=w0_ch[i][:],
                                  in_=w[:, i * WCH:(i + 1) * WCH])
            blab_sb = cpool.tile([NPRED, CW], bf16, tag="blab")
            nc.sync.dma_start(out=blab_sb[:], in_=blab[:])
            bgb_sb = cpool.tile([128, NPRED], bf16, tag="bgb")
            nc.sync.dma_start(out=bgb_sb[:], in_=bgb[:])
            srct_sb = cpool.tile([128, NBLK * 2 * EPB], bf16, tag="srct")
            nc.sync.dma_start(out=srct_sb[:], in_=srct[:])
            srco_sb = cpool.tile([EPB, NBLK * 2 * 128], bf16, tag="srco")
            nc.sync.dma_start(out=srco_sb[:], in_=srco[:])
            tgto_sb = cpool.tile([EPB, NBLK * 2 * 128], bf16, tag="tgto")
            nc.sync.dma_start(out=tgto_sb[:], in_=tgto[:])
            p1h_sb = cpool.tile([EPB, NBLK * NPRED], bf16, tag="p1h")
            nc.sync.dma_start(out=p1h_sb[:], in_=p1h[:])
            p1hs_sb = cpool.tile([128, NPRED], bf16, tag="p1hs")
            nc.sync.dma_start(out=p1hs_sb[:], in_=p1hs[:])
            ident_sb = cpool.tile([128, 128], bf16, tag="ident")
            nc.sync.dma_start(out=ident_sb[:], in_=ident[:])
            for k in (1, 2):
                nc.sync.dma_start(out=w_sb_k[k][:],
                                  in_=w[:, k * 32 * CW:(k + 1) * 32 * CW])

            def rft_lhsT(b, d):
                return rft_tiles[b][d // 16][:, (d % 16) * 128:(d % 16 + 1) * 128]

            def h_pass(b, k, h_sb, gf_psum):
                """H_k(b) = RF_b @ W_k (and gf on the k=0 pass)."""
                ph_t = php.tile([128, CW], f32, tag="ph", name=f"ph{b}_{k}")
                for d in range(32):
                    lhsT = rft_lhsT(b, d)
                    if k == 0:
                        w_rhs = w0_ch[d // 4][:, (d % 4) * CW:(d % 4 + 1) * CW]
                    else:
                        w_rhs = w_sb_k[k][:, d * CW:(d + 1) * CW]
                    nc.tensor.matmul(ph_t[:], lhsT, w_rhs,
                                     start=(d == 0), stop=(d == 31))
                    if k == 0:
                        nc.tensor.matmul(gf_psum[:], lhsT,
                                         wg_sb[:, d * 3:(d + 1) * 3],
                                         start=(d == 0), stop=(d == 31))
                hk = hpool.tile([128, CW], bf16, tag=f"h{k}", name=f"h{b}_{k}")
                nc.vector.tensor_copy(out=hk[:], in_=ph_t[:])
                h_sb[(b, k)] = hk

            def build(b, gf_sb, mtgt):
                """Gates -> block-diagonal M_k (lhsT form) and G^T."""
                sig = []
                for k in range(2):
                    sg = spool.tile([128, NPRED], bf16, tag=f"sig{k}",
                                    name=f"sig{b}_{k}")
                    nc.scalar.activation(sg[:], bgb_sb[:], AF.Sigmoid,
                                         bias=gf_sb[:, k:k + 1])
                    sig.append(sg)
                g2 = spool.tile([128, 1], f32, tag="g2", name=f"g2_{b}")
                nc.scalar.activation(g2[:], bgb_sb[:, 0:1], AF.Sigmoid,
                                     bias=gf_sb[:, 2:3])

                mt_sb = spool.tile([128, 3 * 128], bf16, tag="mt",
                                   name=f"mt{b}")
                pgt_t = pgtp.tile([NPRED, 128], f32, tag="pgt", name=f"pgt{b}")
                for k in range(2):
                    prg_t = prgp.tile([EPB, NPRED], f32, tag="prg",
                                      name=f"prg{b}_{k}")
                    nc.tensor.matmul(
                        prg_t[:],
                        srct_sb[:, (b * 2 + k) * EPB:(b * 2 + k + 1) * EPB],
                        sig[k][:], start=True, stop=True)
                    pg = spool.tile([EPB, NPRED], bf16, tag="pg",
                                    name=f"pg{b}_{k}")
                    nc.vector.tensor_mul(
                        pg[:], prg_t[:],
                        p1h_sb[:, b * NPRED:(b + 1) * NPRED])
                    nc.tensor.matmul(
                        pgt_t[:], pg[:],
                        tgto_sb[:, (b * 2 + k) * 128:(b * 2 + k + 1) * 128],
                        start=(k == 0), stop=False)
                    gcol = spool.tile([EPB, 1], f32, tag="gcol",
                                      name=f"gcol{b}_{k}")
                    nc.vector.tensor_reduce(gcol[:], pg[:],
                                            axis=mybir.AxisListType.X,
                                            op=ALU.add)
                    srcg = spool.tile([EPB, 128], bf16, tag="srcg",
                                      name=f"srcg{b}_{k}")
                    nc.vector.tensor_scalar_mul(
                        srcg[:],
                        srco_sb[:, (b * 2 + k) * 128:(b * 2 + k + 1) * 128],
                        gcol[:])
                    pmt_t = pmtp.tile([128, 128], f32, tag="pmt",
                                      name=f"pmt{b}_{k}")
                    nc.tensor.matmul(
                        pmt_t[:], srcg[:],
                        tgto_sb[:, (b * 2 + k) * 128:(b * 2 + k + 1) * 128],
                        start=True, stop=True)
                    nc.vector.tensor_copy(out=mt_sb[:, k * 128:(k + 1) * 128],
                                          in_=pmt_t[:])
                # self-loop: M_2 = diag(g2); G row 0 += g2
                pg2 = spool.tile([128, NPRED], bf16, tag="pg2", name=f"pg2_{b}")
                nc.vector.tensor_scalar_mul(pg2[:], p1hs_sb[:], g2[:])
                nc.tensor.matmul(pgt_t[:], pg2[:], ident_sb[:],
                                 start=False, stop=True)
                gt_sb = spool.tile([NPRED, 128], bf16, tag="gt", name=f"gt{b}")
                nc.vector.tensor_copy(out=gt_sb[:], in_=pgt_t[:])
                nc.vector.tensor_scalar_mul(mt_sb[:, 2 * 128:3 * 128],
                                            ident_sb[:], g2[:])
                mtgt[b] = (mt_sb, gt_sb)

            def stage3(b, h_sb, mtgt):
                mt_sb, gt_sb = mtgt[b]
                pout_t = poutp.tile([128, CW], f32, tag="pout", name=f"po{b}")
                for k in range(3):
                    nc.tensor.matmul(pout_t[:],
                                     mt_sb[:, k * 128:(k + 1) * 128],
                                     h_sb[(b, k)][:],
                                     start=(k == 0), stop=False)
                nc.tensor.matmul(pout_t[:], gt_sb[:], blab_sb[:],
                                 start=False, stop=True)
                out_sb = opool.tile([128, CW], f32, tag="out", name=f"ob{b}")
                nc.scalar.activation(out_sb[:], pout_t[:], AF.Relu)
                nc.sync.dma_start(out=out[b], in_=out_sb[:])

            # Software pipeline over groups of GRP blocks: all k=0 passes
            # first (gives the W[k=1,2] DMAs compute to hide behind), then
            # builds, then k=1, then k=2 + stage3.
            for g0 in range(0, NBLK, GRP):
                grp = list(range(g0, min(g0 + GRP, NBLK)))
                h_sb, gf_tiles, mtgt = {}, {}, {}
                for b in grp:
                    pgf_t = pgfp.tile([128, 3], f32, tag="pgf", name=f"pgf{b}")
                    h_pass(b, 0, h_sb, pgf_t)
                    gf_sb = spool.tile([128, 3], f32, tag="gf", name=f"gf{b}")
                    nc.vector.tensor_copy(out=gf_sb[:], in_=pgf_t[:])
                    gf_tiles[b] = gf_sb
                # prefetch next group's RF^T while k=1/k=2 run
                for b in range(g0 + GRP, min(g0 + 2 * GRP, NBLK)):
                    rft_tiles[b] = _load_rft(b)
                for b in grp:
                    build(b, gf_tiles[b], mtgt)
                for b in grp:
                    h_pass(b, 1, h_sb, None)
                for b in grp:
                    h_pass(b, 2, h_sb, None)
                    stage3(b, h_sb, mtgt)
                for b in grp:
                    del rft_tiles[b]

    nc.compile()
    return nc


def _host_prep(inputs):
    rf = np.asarray(inputs["region_feats"], dtype=np.float32)
    W = np.asarray(inputs["W_conv"], dtype=np.float32)
    Wg = np.asarray(inputs["W_g"], dtype=np.float32)
    blab = np.asarray(inputs["b_lab"], dtype=np.float32)
    bglab = np.asarray(inputs["b_glab"], dtype=np.float32)
    rels = np.asarray(inputs["rels"])
    preds = np.asarray(inputs["pred_classes"])

    rels_r = rels.reshape(N_IMG, RPI, 3)[:, :NUM_REL].reshape(-1, 3)
    preds_r = preds.reshape(N_IMG, RPI)[:, :NUM_REL].reshape(-1)

    # RF^T tiles: rft_h[b, p, d*128+j] = RF[b*128+j, d*128+p]
    rft_h = np.ascontiguousarray(
        rf.T.reshape(32, 128, NBLK, 128).transpose(2, 1, 0, 3), dtype=BF
    ).reshape(NBLK, 128, 32 * 128)

    # W slices per core: w_h[p, ((k*32+d)*CW)+j] = W[d*128+p, k*D + c*CW + j]
    Wr = W.reshape(32, 128, 3, NCORES, CW)
    w_cores = [
        np.ascontiguousarray(Wr[:, :, :, c, :].transpose(1, 2, 0, 3),
                             dtype=BF).reshape(128, 3 * 32 * CW)
        for c in range(NCORES)
    ]
    wg_h = np.ascontiguousarray(
        Wg.reshape(32, 128, 3).transpose(1, 0, 2), dtype=BF
    ).reshape(128, 32 * 3)
    blab_cores = [
        np.ascontiguousarray(blab[:, c * CW:(c + 1) * CW], dtype=BF)
        for c in range(NCORES)
    ]
    bgb_h = np.ascontiguousarray(
        np.repeat(bglab.reshape(1, NPRED), 128, axis=0), dtype=BF)

    srct_h = np.zeros((128, NBLK * 2 * EPB), np.float32)
    srco_h = np.zeros((EPB, NBLK * 2 * 128), np.float32)
    tgto_h = np.zeros((EPB, NBLK * 2 * 128), np.float32)
    p1h_h = np.zeros((EPB, NBLK * NPRED), np.float32)
    e = np.arange(EPB)
    for b in range(NBLK):
        eb = rels_r[b * EPB:(b + 1) * EPB]
        pb = preds_r[b * EPB:(b + 1) * EPB]
        s = eb[:, 1] - b * 128
        o = eb[:, 2] - b * 128
        # k=0: obj -> subj (src=o, tgt=s); k=1: subj -> obj (src=s, tgt=o)
        srct_h[o, (b * 2 + 0) * EPB + e] = 1.0
        srct_h[s, (b * 2 + 1) * EPB + e] = 1.0
        srco_h[e, (b * 2 + 0) * 128 + o] = 1.0
        srco_h[e, (b * 2 + 1) * 128 + s] = 1.0
        tgto_h[e, (b * 2 + 0) * 128 + s] = 1.0
        tgto_h[e, (b * 2 + 1) * 128 + o] = 1.0
        p1h_h[e, b * NPRED + pb] = 1.0
    p1hs_h = np.zeros((128, NPRED), np.float32)
    p1hs_h[:, 0] = 1.0

    shared = {
        "rft": rft_h,
        "wg": wg_h,
        "bgb": bgb_h,
        "srct": srct_h.astype(BF),
        "srco": srco_h.astype(BF),
        "tgto": tgto_h.astype(BF),
        "p1h": p1h_h.astype(BF),
        "p1hs": p1hs_h.astype(BF),
        "ident": np.eye(128, dtype=np.float32).astype(BF),
    }
    in_maps = []
    for c in range(NCORES):
        m = dict(shared)
        m["w"] = w_cores[c]
        m["blab"] = blab_cores[c]
        in_maps.append(m)
    return in_maps


def _rels_are_blocked(rels):
    """Check each image's relations reference only that image's regions."""
    rels = np.asarray(rels)
    if rels.shape != (N_IMG * RPI, 3):
        return False
    rels_r = rels.reshape(N_IMG, RPI, 3)[:, :NUM_REL]
    img = np.arange(N_IMG)[:, None]
    lo, hi = img * REG, (img + 1) * REG
    so = rels_r[:, :, 1:3]
    return bool(np.all((so >= lo[:, :, None]) & (so < hi[:, :, None])))


def _numpy_fallback(inputs):
    """Reference-equivalent host computation (only used if the per-image
    relation structure assumption is violated)."""
    rf = np.asarray(inputs["region_feats"], dtype=np.float32)
    W = np.asarray(inputs["W_conv"], dtype=np.float32)
    Wg = np.asarray(inputs["W_g"], dtype=np.float32)
    blab = np.asarray(inputs["b_lab"], dtype=np.float32)
    bglab = np.asarray(inputs["b_glab"], dtype=np.float32)
    rels = np.asarray(inputs["rels"])
    preds = np.asarray(inputs["pred_classes"])
    rels_r = rels.reshape(N_IMG, RPI, 3)[:, :NUM_REL].reshape(-1, 3)
    preds_r = preds.reshape(N_IMG, RPI)[:, :NUM_REL].reshape(-1)
    nf = (rf @ W).reshape(-1, D)
    gfe = (rf @ Wg).reshape(-1)
    s, o = rels_r[:, 1], rels_r[:, 2]
    self_ids = np.arange(N)
    idx = np.concatenate([o * 3 + 0, s * 3 + 1, self_ids * 3 + 2])
    pr = np.concatenate([preds_r, preds_r, np.zeros(N, preds_r.dtype)])
    tgt = np.concatenate([s, o, self_ids])
    gate = 1.0 / (1.0 + np.exp(-(gfe[idx] + bglab[pr, 0])))
    msg = gate[:, None] * (nf[idx] + blab[pr])
    upd = np.zeros((N, D), np.float32)
    np.add.at(upd, tgt, msg)
    return np.maximum(upd, 0.0)


def _run(inputs, trace=False):
    from concourse.bass_utils import run_bass_kernel_spmd

    if "nc" not in _prog_cache:
        _prog_cache["nc"] = _build_program()
    nc = _prog_cache["nc"]
    in_maps = _host_prep(inputs)
    res = run_bass_kernel_spmd(nc, in_maps, core_ids=list(range(NCORES)),
                               trace=trace)
    out = np.empty((N, D), np.float32)
    for c in range(NCORES):
        out[:, c * CW:(c + 1) * CW] = (
            np.asarray(res.results[c]["out"]).reshape(N, CW))
    return out, res


def kernel(**inputs):
    if not _rels_are_blocked(inputs["rels"]):
        return _numpy_fallback(inputs)
    out, _ = _run(inputs, trace=False)
    return out


# revision 18
# speedup vs baseline: 1.0660x; 1.0091x over previous
"""GCN message-passing kernel for Trainium2 (8 NeuronCores, SPMD).

Math (matches the reference):
    gf   = RF @ W_g                          (2048, 3)   gate features
    H_k  = RF @ W_k                          (2048, 4096) per edge type k in {0,1,2}
    gate(e) = sigmoid(gf[src_e, k_e] + b_glab[p_e])
    upd[t]  = sum_{e->t} gate(e) * (H_{k_e}[src_e] + b_lab[p_e])
    out  = relu(upd)

Because every image's graph is self-contained (32 regions/image) the
edge aggregation is a block-diagonal linear operator: with 4 images per
128-row block,
    upd = sum_k M_k @ H_k + G @ b_lab
where M_k are (128x128)-block-diagonal gate matrices and G is (2048,81).
M_k / G are built ON DEVICE from gf with one-hot constant matrices (host
only prepares 0/1 index matrices), so all data-dependent FLOPs run on
Trainium.

Sharding: the output D dim (4096) is split 8 ways -> each core computes
all 2048 rows x its 512 columns, holding a (4096 x 3*512) slice of
W_conv.  This avoids replicating the 201MB W_conv read (per-core DMA is
~36MB vs ~210MB for image-sharding).  No collectives needed; host
concatenates the column slices.
"""

import numpy as np
import ml_dtypes

# problem constants (hardcoded per contract)
N_IMG = 64
REG = 32
RPI = 32
NUM_REL = 20
D = 4096
NPRED = 81
N = N_IMG * REG          # 2048
NCORES = 8
CW = D // NCORES         # 512 output cols per core
NBLK = N // 128          # 16 row blocks
IPB = 128 // REG         # 4 images per block
EPB = IPB * NUM_REL      # 80 edges per block per edge type

BF = ml_dtypes.bfloat16

_prog_cache = {}


def _build_program():
    import concourse.bass as bass
    import concourse.tile as tile
    from concourse import bacc, mybir

    bf16 = mybir.dt.bfloat16
    f32 = mybir.dt.float32
    AF = mybir.ActivationFunctionType
    ALU = mybir.AluOpType

    nc = bacc.Bacc("TRN2", target_bir_lowering=False, debug=False,
                   num_devices=NCORES)

    rft = nc.dram_tensor("rft", [NBLK, 128, 32 * 128], bf16, kind="ExternalInput").ap()
    w = nc.dram_tensor("w", [128, 3 * 32 * CW], bf16, kind="ExternalInput").ap()
    wg = nc.dram_tensor("wg", [128, 32 * 3], bf16, kind="ExternalInput").ap()
    blab = nc.dram_tensor("blab", [NPRED, CW], bf16, kind="ExternalInput").ap()
    bgb = nc.dram_tensor("bgb", [128, NPRED], bf16, kind="ExternalInput").ap()
    srct = nc.dram_tensor("srct", [128, NBLK * 2 * EPB], bf16, kind="ExternalInput").ap()
    srco = nc.dram_tensor("srco", [EPB, NBLK * 2 * 128], bf16, kind="ExternalInput").ap()
    tgto = nc.dram_tensor("tgto", [EPB, NBLK * 2 * 128], bf16, kind="ExternalInput").ap()
    p1h = nc.dram_tensor("p1h", [EPB, NBLK * NPRED], bf16, kind="ExternalInput").ap()
    p1hs = nc.dram_tensor("p1hs", [128, NPRED], bf16, kind="ExternalInput").ap()
    ident = nc.dram_tensor("ident", [128, 128], bf16, kind="ExternalInput").ap()
    out = nc.dram_tensor("out", [NBLK, 128, CW], f32, kind="ExternalOutput").ap()

    GRP = 2  # blocks per software-pipeline group
    with tile.TileContext(nc) as tc:
        with (
            tc.tile_pool(name="consts", bufs=1) as cpool,
            tc.tile_pool(name="rft", bufs=2 * GRP) as rpool,
            tc.tile_pool(name="hsb", bufs=GRP) as hpool,
            tc.tile_pool(name="small", bufs=2) as spool,
            tc.tile_pool(name="osb", bufs=2) as opool,
            tc.tile_pool(name="ph", bufs=2, space="PSUM") as php,
            tc.tile_pool(name="pgf", bufs=1, space="PSUM") as pgfp,
            tc.tile_pool(name="prg", bufs=2, space="PSUM") as prgp,
            tc.tile_pool(name="pgt", bufs=1, space="PSUM") as pgtp,
            tc.tile_pool(name="pmt", bufs=1, space="PSUM") as pmtp,
            tc.tile_pool(name="pout", bufs=1, space="PSUM") as poutp,
        ):
            # DMA order matters for startup latency: block 0's first matmuls
            # need wg + the first chunks of W[k=0] + rft[0]'s first half, so
            # those are issued first (W[k=0] split into 8 chunks, rft into
            # halves); the gate constants and W[k=1,2] follow.
            wg_sb = cpool.tile([128, 32 * 3], bf16, tag="wg")
            nc.sync.dma_start(out=wg_sb[:], in_=wg[:])
            w_sb_k = [None]
            for k in (1, 2):
                wk = cpool.tile([128, 32 * CW], bf16, tag=f"w{k}")
                w_sb_k.append(wk)
            WCH = 4 * CW  # w0 chunk: 4 d-tiles
            w0_ch = [cpool.tile([128, WCH], bf16, tag=f"w0c{i}", name=f"w0c{i}")
                     for i in range(8)]
            nc.sync.dma_start(out=w0_ch[0][:], in_=w[:, 0:WCH])

            def _load_rft(b):
                ra = rpool.tile([128, 16 * 128], bf16, tag="rfta",
                                name=f"rfta{b}")
                rb = rpool.tile([128, 16 * 128], bf16, tag="rftb",
                                name=f"rftb{b}")
                nc.sync.dma_start(out=ra[:], in_=rft[b, :, 0:16 * 128])
                nc.sync.dma_start(out=rb[:], in_=rft[b, :, 16 * 128:32 * 128])
                return ra, rb

            rft_tiles = {b: _load_rft(b) for b in range(GRP)}
            for i in range(1, 8):
                nc.sync.dma_start(out=w0_ch[i][:],
                                  in_=w[:, i * WCH:(i + 1) * WCH])
            for k in (1, 2):
                nc.sync.dma_start(out=w_sb_k[k][:],
                                  in_=w[:, k * 32 * CW:(k + 1) * 32 * CW])
            blab_sb = cpool.tile([NPRED, CW], bf16, tag="blab")
            nc.sync.dma_start(out=blab_sb[:], in_=blab[:])
            bgb_sb = cpool.tile([128, NPRED], bf16, tag="bgb")
            nc.sync.dma_start(out=bgb_sb[:], in_=bgb[:])
            srct_sb = cpool.tile([128, NBLK * 2 * EPB], bf16, tag="srct")
            nc.sync.dma_start(out=srct_sb[:], in_=srct[:])
            srco_sb = cpool.tile([EPB, NBLK * 2 * 128], bf16, tag="srco")
            nc.sync.dma_start(out=srco_sb[:], in_=srco[:])
            tgto_sb = cpool.tile([EPB, NBLK * 2 * 128], bf16, tag="tgto")
            nc.sync.dma_start(out=tgto_sb[:], in_=tgto[:])
            p1h_sb = cpool.tile([EPB, NBLK * NPRED], bf16, tag="p1h")
            nc.sync.dma_start(out=p1h_sb[:], in_=p1h[:])
            p1hs_sb = cpool.tile([128, NPRED], bf16, tag="p1hs")
            nc.sync.dma_start(out=p1hs_sb[:], in_=p1hs[:])
            ident_sb = cpool.tile([128, 128], bf16, tag="ident")
            nc.sync.dma_start(out=ident_sb[:], in_=ident[:])

            def rft_lhsT(b, d):
                return rft_tiles[b][d // 16][:, (d % 16) * 128:(d % 16 + 1) * 128]

            def h_pass(b, k, h_sb, gf_psum):
                """H_k(b) = RF_b @ W_k (and gf on the k=0 pass)."""
                ph_t = php.tile([128, CW], f32, tag="ph", name=f"ph{b}_{k}")
                for d in range(32):
                    lhsT = rft_lhsT(b, d)
                    if k == 0:
                        w_rhs = w0_ch[d // 4][:, (d % 4) * CW:(d % 4 + 1) * CW]
                    else:
                        w_rhs = w_sb_k[k][:, d * CW:(d + 1) * CW]
                    nc.tensor.matmul(ph_t[:], lhsT, w_rhs,
                                     start=(d == 0), stop=(d == 31))
                    if k == 0:
                        nc.tensor.matmul(gf_psum[:], lhsT,
                                         wg_sb[:, d * 3:(d + 1) * 3],
                                         start=(d == 0), stop=(d == 31))
                hk = hpool.tile([128, CW], bf16, tag=f"h{k}", name=f"h{b}_{k}")
                nc.vector.tensor_copy(out=hk[:], in_=ph_t[:])
                h_sb[(b, k)] = hk

            def build(b, gf_sb, mtgt):
                """Gates -> block-diagonal M_k (lhsT form) and G^T."""
                sig = []
                for k in range(2):
                    sg = spool.tile([128, NPRED], bf16, tag=f"sig{k}",
                                    name=f"sig{b}_{k}")
                    nc.scalar.activation(sg[:], bgb_sb[:], AF.Sigmoid,
                                         bias=gf_sb[:, k:k + 1])
                    sig.append(sg)
                g2 = spool.tile([128, 1], f32, tag="g2", name=f"g2_{b}")
                nc.scalar.activation(g2[:], bgb_sb[:, 0:1], AF.Sigmoid,
                                     bias=gf_sb[:, 2:3])

                mt_sb = spool.tile([128, 3 * 128], bf16, tag="mt",
                                   name=f"mt{b}")
                pgt_t = pgtp.tile([NPRED, 128], f32, tag="pgt", name=f"pgt{b}")
                for k in range(2):
                    prg_t = prgp.tile([EPB, NPRED], f32, tag="prg",
                                      name=f"prg{b}_{k}")
                    nc.tensor.matmul(
                        prg_t[:],
                        srct_sb[:, (b * 2 + k) * EPB:(b * 2 + k + 1) * EPB],
                        sig[k][:], start=True, stop=True)
                    pg = spool.tile([EPB, NPRED], bf16, tag="pg",
                                    name=f"pg{b}_{k}")
                    nc.vector.tensor_mul(
                        pg[:], prg_t[:],
                        p1h_sb[:, b * NPRED:(b + 1) * NPRED])
                    nc.tensor.matmul(
                        pgt_t[:], pg[:],
                        tgto_sb[:, (b * 2 + k) * 128:(b * 2 + k + 1) * 128],
                        start=(k == 0), stop=False)
                    gcol = spool.tile([EPB, 1], f32, tag="gcol",
                                      name=f"gcol{b}_{k}")
                    nc.vector.tensor_reduce(gcol[:], pg[:],
                                            axis=mybir.AxisListType.X,
                                            op=ALU.add)
                    srcg = spool.tile([EPB, 128], bf16, tag="srcg",
                                      name=f"srcg{b}_{k}")
                    nc.vector.tensor_scalar_mul(
                        srcg[:],
                        srco_sb[:, (b * 2 + k) * 128:(b * 2 + k + 1) * 128],
                        gcol[:])
                    pmt_t = pmtp.tile([128, 128], f32, tag="pmt",
                                      name=f"pmt{b}_{k}")
                    nc.tensor.matmul(
                        pmt_t[:], srcg[:],
                        tgto_sb[:, (b * 2 + k) * 128:(b * 2 + k + 1) * 128],
                        start=True, stop=True)
                    nc.vector.tensor_copy(out=mt_sb[:, k * 128:(k + 1) * 128],
                                          in_=pmt_t[:])
                # self-loop: M_2 = diag(g2); G row 0 += g2
                pg2 = spool.tile([128, NPRED], bf16, tag="pg2", name=f"pg2_{b}")
                nc.vector.tensor_scalar_mul(pg2[:], p1hs_sb[:], g2[:])
                nc.tensor.matmul(pgt_t[:], pg2[:], ident_sb[:],
                                 start=False, stop=True)
                gt_sb = spool.tile([NPRED, 128], bf16, tag="gt", name=f"gt{b}")
                nc.vector.tensor_copy(out=gt_sb[:], in_=pgt_t[:])
                nc.vector.tensor_scalar_mul(mt_sb[:, 2 * 128:3 * 128],
                                            ident_sb[:], g2[:])
                mtgt[b] = (mt_sb, gt_sb)

            def stage3(b, h_sb, mtgt):
                mt_sb, gt_sb = mtgt[b]
                pout_t = poutp.tile([128, CW], f32, tag="pout", name=f"po{b}")
                for k in range(3):
                    nc.tensor.matmul(pout_t[:],
                                     mt_sb[:, k * 128:(k + 1) * 128],
                                     h_sb[(b, k)][:],
                                     start=(k == 0), stop=False)
                nc.tensor.matmul(pout_t[:], gt_sb[:], blab_sb[:],
                                 start=False, stop=True)
                out_sb = opool.tile([128, CW], f32, tag="out", name=f"ob{b}")
                nc.scalar.activation(out_sb[:], pout_t[:], AF.Relu)
                nc.sync.dma_start(out=out[b], in_=out_sb[:])

            # Software pipeline over groups of GRP blocks: all k=0 passes
            # first (gives the W[k=1,2] DMAs compute to hide behind), then
            # builds, then k=1, then k=2 + stage3.
            for g0 in range(0, NBLK, GRP):
                grp = list(range(g0, min(g0 + GRP, NBLK)))
                h_sb, gf_tiles, mtgt = {}, {}, {}
                for b in grp:
                    pgf_t = pgfp.tile([128, 3], f32, tag="pgf", name=f"pgf{b}")
                    h_pass(b, 0, h_sb, pgf_t)
                    gf_sb = spool.tile([128, 3], f32, tag="gf", name=f"gf{b}")
                    nc.vector.tensor_copy(out=gf_sb[:], in_=pgf_t[:])
                    gf_tiles[b] = gf_sb
                # prefetch next group's RF^T while k=1/k=2 run
                for b in range(g0 + GRP, min(g0 + 2 * GRP, NBLK)):
                    rft_tiles[b] = _load_rft(b)
                for b in grp:
                    h_pass(b, 1, h_sb, None)
                for b in grp:
                    build(b, gf_tiles[b], mtgt)
                for b in grp:
                    h_pass(b, 2, h_sb, None)
                    stage3(b, h_sb, mtgt)
                for b in grp:
                    del rft_tiles[b]

    nc.compile()
    return nc


def _host_prep(inputs):
    rf = np.asarray(inputs["region_feats"], dtype=np.float32)
    W = np.asarray(inputs["W_conv"], dtype=np.float32)
    Wg = np.asarray(inputs["W_g"], dtype=np.float32)
    blab = np.asarray(inputs["b_lab"], dtype=np.float32)
    bglab = np.asarray(inputs["b_glab"], dtype=np.float32)
    rels = np.asarray(inputs["rels"])
    preds = np.asarray(inputs["pred_classes"])

    rels_r = rels.reshape(N_IMG, RPI, 3)[:, :NUM_REL].reshape(-1, 3)
    preds_r = preds.reshape(N_IMG, RPI)[:, :NUM_REL].reshape(-1)

    # RF^T tiles: rft_h[b, p, d*128+j] = RF[b*128+j, d*128+p]
    rft_h = np.ascontiguousarray(
        rf.T.reshape(32, 128, NBLK, 128).transpose(2, 1, 0, 3), dtype=BF
    ).reshape(NBLK, 128, 32 * 128)

    # W slices per core: w_h[p, ((k*32+d)*CW)+j] = W[d*128+p, k*D + c*CW + j]
    Wr = W.reshape(32, 128, 3, NCORES, CW)
    w_cores = [
        np.ascontiguousarray(Wr[:, :, :, c, :].transpose(1, 2, 0, 3),
                             dtype=BF).reshape(128, 3 * 32 * CW)
        for c in range(NCORES)
    ]
    wg_h = np.ascontiguousarray(
        Wg.reshape(32, 128, 3).transpose(1, 0, 2), dtype=BF
    ).reshape(128, 32 * 3)
    blab_cores = [
        np.ascontiguousarray(blab[:, c * CW:(c + 1) * CW], dtype=BF)
        for c in range(NCORES)
    ]
    bgb_h = np.ascontiguousarray(
        np.repeat(bglab.reshape(1, NPRED), 128, axis=0), dtype=BF)

    srct_h = np.zeros((128, NBLK * 2 * EPB), np.float32)
    srco_h = np.zeros((EPB, NBLK * 2 * 128), np.float32)
    tgto_h = np.zeros((EPB, NBLK * 2 * 128), np.float32)
    p1h_h = np.zeros((EPB, NBLK * NPRED), np.float32)
    e = np.arange(EPB)
    for b in range(NBLK):
        eb = rels_r[b * EPB:(b + 1) * EPB]
        pb = preds_r[b * EPB:(b + 1) * EPB]
        s = eb[:, 1] - b * 128
        o = eb[:, 2] - b * 128
        # k=0: obj -> subj (src=o, tgt=s); k=1: subj -> obj (src=s, tgt=o)
        srct_h[o, (b * 2 + 0) * EPB + e] = 1.0
        srct_h[s, (b * 2 + 1) * EPB + e] = 1.0
        srco_h[e, (b * 2 + 0) * 128 + o] = 1.0
        srco_h[e, (b * 2 + 1) * 128 + s] = 1.0
        tgto_h[e, (b * 2 + 0) * 128 + s] = 1.0
        tgto_h[e, (b * 2 + 1) * 128 + o] = 1.0
        p1h_h[e, b * NPRED + pb] = 1.0
    p1hs_h = np.zeros((128, NPRED), np.float32)
    p1hs_h[:, 0] = 1.0

    shared = {
        "rft": rft_h,
        "wg": wg_h,
        "bgb": bgb_h,
        "srct": srct_h.astype(BF),
        "srco": srco_h.astype(BF),
        "tgto": tgto_h.astype(BF),
        "p1h": p1h_h.astype(BF),
        "p1hs": p1hs_h.astype(BF),
        "ident": np.eye(128, dtype=np.float32).astype(BF),
    }
    in_maps = []
    for c in range(NCORES):
        m = dict(shared)
        m["w"] = w_cores[c]
        m["blab"] = blab_cores[c]
        in_maps.append(m)
    return in_maps


def _rels_are_blocked(rels):
    """Check each image's relations reference only that image's regions."""
    rels = np.asarray(rels)
    if rels.shape != (N_IMG * RPI, 3):
        return False
    rels_r = rels.reshape(N_IMG, RPI, 3)[:, :NUM_REL]
    img = np.arange(N_IMG)[:, None]
    lo, hi = img * REG, (img + 1) * REG
    so = rels_r[:, :, 1:3]
    return bool(np.all((so >= lo[:, :, None]) & (so < hi[:, :, None])))


def _numpy_fallback(inputs):
    """Reference-equivalent host computation (only used if the per-image
    relation structure assumption is violated)."""
    rf = np.asarray(inputs["region_feats"], dtype=np.float32)
    W = np.asarray(inputs["W_conv"], dtype=np.float32)
    Wg = np.asarray(inputs["W_g"], dtype=np.float32)
    blab = np.asarray(inputs["b_lab"], dtype=np.float32)
    bglab = np.asarray(inputs["b_glab"], dtype=np.float32)
    rels = np.asarray(inputs["rels"])
    preds = np.asarray(inputs["pred_classes"])
    rels_r = rels.reshape(N_IMG, RPI, 3)[:, :NUM_REL].reshape(-1, 3)
    preds_r = preds.reshape(N_IMG, RPI)[:, :NUM_REL].reshape(-1)
    nf = (rf @ W).reshape(-1, D)
    gfe = (rf @ Wg).reshape(-1)
    s, o = rels_r[:, 1], rels_r[:, 2]
    self_ids = np.arange(N)
    idx = np.concatenate([o * 3 + 0, s * 3 + 1, self_ids * 3 + 2])
    pr = np.concatenate([preds_r, preds_r, np.zeros(N, preds_r.dtype)])
    tgt = np.concatenate([s, o, self_ids])
    gate = 1.0 / (1.0 + np.exp(-(gfe[idx] + bglab[pr, 0])))
    msg = gate[:, None] * (nf[idx] + blab[pr])
    upd = np.zeros((N, D), np.float32)
    np.add.at(upd, tgt, msg)
    return np.maximum(upd, 0.0)


def _run(inputs, trace=False):
    from concourse.bass_utils import run_bass_kernel_spmd

    if "nc" not in _prog_cache:
        _prog_cache["nc"] = _build_program()
    nc = _prog_cache["nc"]
    in_maps = _host_prep(inputs)
    res = run_bass_kernel_spmd(nc, in_maps, core_ids=list(range(NCORES)),
                               trace=trace)
    out = np.empty((N, D), np.float32)
    for c in range(NCORES):
        out[:, c * CW:(c + 1) * CW] = (
            np.asarray(res.results[c]["out"]).reshape(N, CW))
    return out, res


def kernel(**inputs):
    if not _rels_are_blocked(inputs["rels"]):
        return _numpy_fallback(inputs)
    out, _ = _run(inputs, trace=False)
    return out


# revision 19
# speedup vs baseline: 1.0928x; 1.0251x over previous
"""GCN message-passing kernel for Trainium2 (8 NeuronCores, SPMD).

Math (matches the reference):
    gf   = RF @ W_g                          (2048, 3)   gate features
    H_k  = RF @ W_k                          (2048, 4096) per edge type k in {0,1,2}
    gate(e) = sigmoid(gf[src_e, k_e] + b_glab[p_e])
    upd[t]  = sum_{e->t} gate(e) * (H_{k_e}[src_e] + b_lab[p_e])
    out  = relu(upd)

Because every image's graph is self-contained (32 regions/image) the
edge aggregation is a block-diagonal linear operator: with 4 images per
128-row block,
    upd = sum_k M_k @ H_k + G @ b_lab
where M_k are (128x128)-block-diagonal gate matrices and G is (2048,81).
M_k / G are built ON DEVICE from gf with one-hot constant matrices (host
only prepares 0/1 index matrices), so all data-dependent FLOPs run on
Trainium.

Sharding: the output D dim (4096) is split 8 ways -> each core computes
all 2048 rows x its 512 columns, holding a (4096 x 3*512) slice of
W_conv.  This avoids replicating the 201MB W_conv read (per-core DMA is
~36MB vs ~210MB for image-sharding).  No collectives needed; host
concatenates the column slices.
"""

import numpy as np
import ml_dtypes

# problem constants (hardcoded per contract)
N_IMG = 64
REG = 32
RPI = 32
NUM_REL = 20
D = 4096
NPRED = 81
N = N_IMG * REG          # 2048
NCORES = 8
CW = D // NCORES         # 512 output cols per core
NBLK = N // 128          # 16 row blocks
IPB = 128 // REG         # 4 images per block
EPB = IPB * NUM_REL      # 80 edges per block per edge type

BF = ml_dtypes.bfloat16

_prog_cache = {}


def _build_program():
    import concourse.bass as bass
    import concourse.tile as tile
    from concourse import bacc, mybir

    bf16 = mybir.dt.bfloat16
    f32 = mybir.dt.float32
    AF = mybir.ActivationFunctionType
    ALU = mybir.AluOpType

    nc = bacc.Bacc("TRN2", target_bir_lowering=False, debug=False,
                   num_devices=NCORES)

    rft = nc.dram_tensor("rft", [NBLK, 128, 32 * 128], bf16, kind="ExternalInput").ap()
    w = nc.dram_tensor("w", [128, 3 * 32 * CW], bf16, kind="ExternalInput").ap()
    wg = nc.dram_tensor("wg", [128, 32 * 3], bf16, kind="ExternalInput").ap()
    blab = nc.dram_tensor("blab", [NPRED, CW], bf16, kind="ExternalInput").ap()
    bgb = nc.dram_tensor("bgb", [128, NPRED], bf16, kind="ExternalInput").ap()
    srct = nc.dram_tensor("srct", [128, NBLK * 2 * EPB], bf16, kind="ExternalInput").ap()
    srco = nc.dram_tensor("srco", [EPB, NBLK * 2 * 128], bf16, kind="ExternalInput").ap()
    tgto = nc.dram_tensor("tgto", [EPB, NBLK * 2 * 128], bf16, kind="ExternalInput").ap()
    p1h = nc.dram_tensor("p1h", [EPB, NBLK * NPRED], bf16, kind="ExternalInput").ap()
    p1hs = nc.dram_tensor("p1hs", [128, NPRED], bf16, kind="ExternalInput").ap()
    ident = nc.dram_tensor("ident", [128, 128], bf16, kind="ExternalInput").ap()
    out = nc.dram_tensor("out", [NBLK, 128, CW], f32, kind="ExternalOutput").ap()

    GRP = 3  # blocks per software-pipeline group
    with tile.TileContext(nc) as tc:
        with (
            tc.tile_pool(name="consts", bufs=1) as cpool,
            tc.tile_pool(name="rft", bufs=2 * GRP) as rpool,
            tc.tile_pool(name="hsb", bufs=GRP) as hpool,
            tc.tile_pool(name="small", bufs=2) as spool,
            tc.tile_pool(name="osb", bufs=2) as opool,
            tc.tile_pool(name="ph", bufs=2, space="PSUM") as php,
            tc.tile_pool(name="pgf", bufs=1, space="PSUM") as pgfp,
            tc.tile_pool(name="prg", bufs=2, space="PSUM") as prgp,
            tc.tile_pool(name="pgt", bufs=1, space="PSUM") as pgtp,
            tc.tile_pool(name="pmt", bufs=1, space="PSUM") as pmtp,
            tc.tile_pool(name="pout", bufs=1, space="PSUM") as poutp,
        ):
            # DMA order matters for startup latency: block 0's first matmuls
            # need wg + the first chunks of W[k=0] + rft[0]'s first half, so
            # those are issued first (W[k=0] split into 8 chunks, rft into
            # halves); the gate constants and W[k=1,2] follow.
            wg_sb = cpool.tile([128, 32 * 3], bf16, tag="wg")
            nc.sync.dma_start(out=wg_sb[:], in_=wg[:])
            w_sb_k = [None]
            for k in (1, 2):
                wk = cpool.tile([128, 32 * CW], bf16, tag=f"w{k}")
                w_sb_k.append(wk)
            WCH = 4 * CW  # w0 chunk: 4 d-tiles
            w0_ch = [cpool.tile([128, WCH], bf16, tag=f"w0c{i}", name=f"w0c{i}")
                     for i in range(8)]
            nc.sync.dma_start(out=w0_ch[0][:], in_=w[:, 0:WCH])

            def _load_rft(b):
                ra = rpool.tile([128, 16 * 128], bf16, tag="rfta",
                                name=f"rfta{b}")
                rb = rpool.tile([128, 16 * 128], bf16, tag="rftb",
                                name=f"rftb{b}")
                nc.sync.dma_start(out=ra[:], in_=rft[b, :, 0:16 * 128])
                nc.sync.dma_start(out=rb[:], in_=rft[b, :, 16 * 128:32 * 128])
                return ra, rb

            rft_tiles = {b: _load_rft(b) for b in range(GRP)}
            for i in range(1, 8):
                nc.sync.dma_start(out=w0_ch[i][:],
                                  in_=w[:, i * WCH:(i + 1) * WCH])
            for k in (1, 2):
                nc.sync.dma_start(out=w_sb_k[k][:],
                                  in_=w[:, k * 32 * CW:(k + 1) * 32 * CW])
            blab_sb = cpool.tile([NPRED, CW], bf16, tag="blab")
            nc.sync.dma_start(out=blab_sb[:], in_=blab[:])
            bgb_sb = cpool.tile([128, NPRED], bf16, tag="bgb")
            nc.sync.dma_start(out=bgb_sb[:], in_=bgb[:])
            srct_sb = cpool.tile([128, NBLK * 2 * EPB], bf16, tag="srct")
            nc.sync.dma_start(out=srct_sb[:], in_=srct[:])
            srco_sb = cpool.tile([EPB, NBLK * 2 * 128], bf16, tag="srco")
            nc.sync.dma_start(out=srco_sb[:], in_=srco[:])
            tgto_sb = cpool.tile([EPB, NBLK * 2 * 128], bf16, tag="tgto")
            nc.sync.dma_start(out=tgto_sb[:], in_=tgto[:])
            p1h_sb = cpool.tile([EPB, NBLK * NPRED], bf16, tag="p1h")
            nc.sync.dma_start(out=p1h_sb[:], in_=p1h[:])
            p1hs_sb = cpool.tile([128, NPRED], bf16, tag="p1hs")
            nc.sync.dma_start(out=p1hs_sb[:], in_=p1hs[:])
            ident_sb = cpool.tile([128, 128], bf16, tag="ident")
            nc.sync.dma_start(out=ident_sb[:], in_=ident[:])

            def rft_lhsT(b, d):
                return rft_tiles[b][d // 16][:, (d % 16) * 128:(d % 16 + 1) * 128]

            def h_pass(b, k, h_sb, gf_psum):
                """H_k(b) = RF_b @ W_k (and gf on the k=0 pass)."""
                ph_t = php.tile([128, CW], f32, tag="ph", name=f"ph{b}_{k}")
                for d in range(32):
                    lhsT = rft_lhsT(b, d)
                    if k == 0:
                        w_rhs = w0_ch[d // 4][:, (d % 4) * CW:(d % 4 + 1) * CW]
                    else:
                        w_rhs = w_sb_k[k][:, d * CW:(d + 1) * CW]
                    nc.tensor.matmul(ph_t[:], lhsT, w_rhs,
                                     start=(d == 0), stop=(d == 31))
                    if k == 0:
                        nc.tensor.matmul(gf_psum[:], lhsT,
                                         wg_sb[:, d * 3:(d + 1) * 3],
                                         start=(d == 0), stop=(d == 31))
                hk = hpool.tile([128, CW], bf16, tag=f"h{k}", name=f"h{b}_{k}")
                nc.vector.tensor_copy(out=hk[:], in_=ph_t[:])
                h_sb[(b, k)] = hk

            def build(b, gf_sb, mtgt):
                """Gates -> block-diagonal M_k (lhsT form) and G^T."""
                sig = []
                for k in range(2):
                    sg = spool.tile([128, NPRED], bf16, tag=f"sig{k}",
                                    name=f"sig{b}_{k}")
                    nc.scalar.activation(sg[:], bgb_sb[:], AF.Sigmoid,
                                         bias=gf_sb[:, k:k + 1])
                    sig.append(sg)
                g2 = spool.tile([128, 1], f32, tag="g2", name=f"g2_{b}")
                nc.scalar.activation(g2[:], bgb_sb[:, 0:1], AF.Sigmoid,
                                     bias=gf_sb[:, 2:3])

                mt_sb = spool.tile([128, 3 * 128], bf16, tag="mt",
                                   name=f"mt{b}")
                pgt_t = pgtp.tile([NPRED, 128], f32, tag="pgt", name=f"pgt{b}")
                for k in range(2):
                    prg_t = prgp.tile([EPB, NPRED], f32, tag="prg",
                                      name=f"prg{b}_{k}")
                    nc.tensor.matmul(
                        prg_t[:],
                        srct_sb[:, (b * 2 + k) * EPB:(b * 2 + k + 1) * EPB],
                        sig[k][:], start=True, stop=True)
                    pg = spool.tile([EPB, NPRED], bf16, tag="pg",
                                    name=f"pg{b}_{k}")
                    nc.vector.tensor_mul(
                        pg[:], prg_t[:],
                        p1h_sb[:, b * NPRED:(b + 1) * NPRED])
                    nc.tensor.matmul(
                        pgt_t[:], pg[:],
                        tgto_sb[:, (b * 2 + k) * 128:(b * 2 + k + 1) * 128],
                        start=(k == 0), stop=False)
                    gcol = spool.tile([EPB, 1], f32, tag="gcol",
                                      name=f"gcol{b}_{k}")
                    nc.vector.tensor_reduce(gcol[:], pg[:],
                                            axis=mybir.AxisListType.X,
                                            op=ALU.add)
                    srcg = spool.tile([EPB, 128], bf16, tag="srcg",
                                      name=f"srcg{b}_{k}")
                    nc.vector.tensor_scalar_mul(
                        srcg[:],
                        srco_sb[:, (b * 2 + k) * 128:(b * 2 + k + 1) * 128],
                        gcol[:])
                    pmt_t = pmtp.tile([128, 128], f32, tag="pmt",
                                      name=f"pmt{b}_{k}")
                    nc.tensor.matmul(
                        pmt_t[:], srcg[:],
                        tgto_sb[:, (b * 2 + k) * 128:(b * 2 + k + 1) * 128],
                        start=True, stop=True)
                    nc.vector.tensor_copy(out=mt_sb[:, k * 128:(k + 1) * 128],
                                          in_=pmt_t[:])
                # self-loop: M_2 = diag(g2); G row 0 += g2
                pg2 = spool.tile([128, NPRED], bf16, tag="pg2", name=f"pg2_{b}")
                nc.vector.tensor_scalar_mul(pg2[:], p1hs_sb[:], g2[:])
                nc.tensor.matmul(pgt_t[:], pg2[:], ident_sb[:],
                                 start=False, stop=True)
                gt_sb = spool.tile([NPRED, 128], bf16, tag="gt", name=f"gt{b}")
                nc.vector.tensor_copy(out=gt_sb[:], in_=pgt_t[:])
                nc.vector.tensor_scalar_mul(mt_sb[:, 2 * 128:3 * 128],
                                            ident_sb[:], g2[:])
                mtgt[b] = (mt_sb, gt_sb)

            def stage3(b, h_sb, mtgt):
                mt_sb, gt_sb = mtgt[b]
                pout_t = poutp.tile([128, CW], f32, tag="pout", name=f"po{b}")
                for k in range(3):
                    nc.tensor.matmul(pout_t[:],
                                     mt_sb[:, k * 128:(k + 1) * 128],
                                     h_sb[(b, k)][:],
                                     start=(k == 0), stop=False)
                nc.tensor.matmul(pout_t[:], gt_sb[:], blab_sb[:],
                                 start=False, stop=True)
                out_sb = opool.tile([128, CW], f32, tag="out", name=f"ob{b}")
                nc.scalar.activation(out_sb[:], pout_t[:], AF.Relu)
                nc.sync.dma_start(out=out[b], in_=out_sb[:])

            # Software pipeline over groups of GRP blocks: all k=0 passes
            # first (gives the W[k=1,2] DMAs compute to hide behind), then
            # builds, then k=1, then k=2 + stage3.
            for g0 in range(0, NBLK, GRP):
                grp = list(range(g0, min(g0 + GRP, NBLK)))
                h_sb, gf_tiles, mtgt = {}, {}, {}
                for b in grp:
                    pgf_t = pgfp.tile([128, 3], f32, tag="pgf", name=f"pgf{b}")
                    h_pass(b, 0, h_sb, pgf_t)
                    gf_sb = spool.tile([128, 3], f32, tag="gf", name=f"gf{b}")
                    nc.vector.tensor_copy(out=gf_sb[:], in_=pgf_t[:])
                    gf_tiles[b] = gf_sb
                # prefetch next group's RF^T while k=1/k=2 run
                for b in range(g0 + GRP, min(g0 + 2 * GRP, NBLK)):
                    rft_tiles[b] = _load_rft(b)
                for b in grp:
                    h_pass(b, 1, h_sb, None)
                for b in grp:
                    build(b, gf_tiles[b], mtgt)
                for b in grp:
                    h_pass(b, 2, h_sb, None)
                    stage3(b, h_sb, mtgt)
                for b in grp:
                    del rft_tiles[b]

    nc.compile()
    return nc


def _host_prep(inputs):
    rf = np.asarray(inputs["region_feats"], dtype=np.float32)
    W = np.asarray(inputs["W_conv"], dtype=np.float32)
    Wg = np.asarray(inputs["W_g"], dtype=np.float32)
    blab = np.asarray(inputs["b_lab"], dtype=np.float32)
    bglab = np.asarray(inputs["b_glab"], dtype=np.float32)
    rels = np.asarray(inputs["rels"])
    preds = np.asarray(inputs["pred_classes"])

    rels_r = rels.reshape(N_IMG, RPI, 3)[:, :NUM_REL].reshape(-1, 3)
    preds_r = preds.reshape(N_IMG, RPI)[:, :NUM_REL].reshape(-1)

    # RF^T tiles: rft_h[b, p, d*128+j] = RF[b*128+j, d*128+p]
    rft_h = np.ascontiguousarray(
        rf.T.reshape(32, 128, NBLK, 128).transpose(2, 1, 0, 3), dtype=BF
    ).reshape(NBLK, 128, 32 * 128)

    # W slices per core: w_h[p, ((k*32+d)*CW)+j] = W[d*128+p, k*D + c*CW + j]
    Wr = W.reshape(32, 128, 3, NCORES, CW)
    w_cores = [
        np.ascontiguousarray(Wr[:, :, :, c, :].transpose(1, 2, 0, 3),
                             dtype=BF).reshape(128, 3 * 32 * CW)
        for c in range(NCORES)
    ]
    wg_h = np.ascontiguousarray(
        Wg.reshape(32, 128, 3).transpose(1, 0, 2), dtype=BF
    ).reshape(128, 32 * 3)
    blab_cores = [
        np.ascontiguousarray(blab[:, c * CW:(c + 1) * CW], dtype=BF)
        for c in range(NCORES)
    ]
    bgb_h = np.ascontiguousarray(
        np.repeat(bglab.reshape(1, NPRED), 128, axis=0), dtype=BF)

    srct_h = np.zeros((128, NBLK * 2 * EPB), np.float32)
    srco_h = np.zeros((EPB, NBLK * 2 * 128), np.float32)
    tgto_h = np.zeros((EPB, NBLK * 2 * 128), np.float32)
    p1h_h = np.zeros((EPB, NBLK * NPRED), np.float32)
    e = np.arange(EPB)
    for b in range(NBLK):
        eb = rels_r[b * EPB:(b + 1) * EPB]
        pb = preds_r[b * EPB:(b + 1) * EPB]
        s = eb[:, 1] - b * 128
        o = eb[:, 2] - b * 128
        # k=0: obj -> subj (src=o, tgt=s); k=1: subj -> obj (src=s, tgt=o)
        srct_h[o, (b * 2 + 0) * EPB + e] = 1.0
        srct_h[s, (b * 2 + 1) * EPB + e] = 1.0
        srco_h[e, (b * 2 + 0) * 128 + o] = 1.0
        srco_h[e, (b * 2 + 1) * 128 + s] = 1.0
        tgto_h[e, (b * 2 + 0) * 128 + s] = 1.0
        tgto_h[e, (b * 2 + 1) * 128 + o] = 1.0
        p1h_h[e, b * NPRED + pb] = 1.0
    p1hs_h = np.zeros((128, NPRED), np.float32)
    p1hs_h[:, 0] = 1.0

    shared = {
        "rft": rft_h,
        "wg": wg_h,
        "bgb": bgb_h,
        "srct": srct_h.astype(BF),
        "srco": srco_h.astype(BF),
        "tgto": tgto_h.astype(BF),
        "p1h": p1h_h.astype(BF),
        "p1hs": p1hs_h.astype(BF),
        "ident": np.eye(128, dtype=np.float32).astype(BF),
    }
    in_maps = []
    for c in range(NCORES):
        m = dict(shared)
        m["w"] = w_cores[c]
        m["blab"] = blab_cores[c]
        in_maps.append(m)
    return in_maps


def _rels_are_blocked(rels):
    """Check each image's relations reference only that image's regions."""
    rels = np.asarray(rels)
    if rels.shape != (N_IMG * RPI, 3):
        return False
    rels_r = rels.reshape(N_IMG, RPI, 3)[:, :NUM_REL]
    img = np.arange(N_IMG)[:, None]
    lo, hi = img * REG, (img + 1) * REG
    so = rels_r[:, :, 1:3]
    return bool(np.all((so >= lo[:, :, None]) & (so < hi[:, :, None])))


def _numpy_fallback(inputs):
    """Reference-equivalent host computation (only used if the per-image
    relation structure assumption is violated)."""
    rf = np.asarray(inputs["region_feats"], dtype=np.float32)
    W = np.asarray(inputs["W_conv"], dtype=np.float32)
    Wg = np.asarray(inputs["W_g"], dtype=np.float32)
    blab = np.asarray(inputs["b_lab"], dtype=np.float32)
    bglab = np.asarray(inputs["b_glab"], dtype=np.float32)
    rels = np.asarray(inputs["rels"])
    preds = np.asarray(inputs["pred_classes"])
    rels_r = rels.reshape(N_IMG, RPI, 3)[:, :NUM_REL].reshape(-1, 3)
    preds_r = preds.reshape(N_IMG, RPI)[:, :NUM_REL].reshape(-1)
    nf = (rf @ W).reshape(-1, D)
    gfe = (rf @ Wg).reshape(-1)
    s, o = rels_r[:, 1], rels_r[:, 2]
    self_ids = np.arange(N)
    idx = np.concatenate([o * 3 + 0, s * 3 + 1, self_ids * 3 + 2])
    pr = np.concatenate([preds_r, preds_r, np.zeros(N, preds_r.dtype)])
    tgt = np.concatenate([s, o, self_ids])
    gate = 1.0 / (1.0 + np.exp(-(gfe[idx] + bglab[pr, 0])))
    msg = gate[:, None] * (nf[idx] + blab[pr])
    upd = np.zeros((N, D), np.float32)
    np.add.at(upd, tgt, msg)
    return np.maximum(upd, 0.0)


def _run(inputs, trace=False):
    from concourse.bass_utils import run_bass_kernel_spmd

    if "nc" not in _prog_cache:
        _prog_cache["nc"] = _build_program()
    nc = _prog_cache["nc"]
    in_maps = _host_prep(inputs)
    res = run_bass_kernel_spmd(nc, in_maps, core_ids=list(range(NCORES)),
                               trace=trace)
    out = np.empty((N, D), np.float32)
    for c in range(NCORES):
        out[:, c * CW:(c + 1) * CW] = (
            np.asarray(res.results[c]["out"]).reshape(N, CW))
    return out, res


def kernel(**inputs):
    if not _rels_are_blocked(inputs["rels"]):
        return _numpy_fallback(inputs)
    out, _ = _run(inputs, trace=False)
    return out
